# revision 3
# baseline (speedup 1.0000x reference)
"""Adaptive Spectral Block on 8 TRN2 NeuronCores (data-parallel over batch).

v2 of the matmul-FFT kernel (N = 4097 = 241*17 Cooley-Tukey):
  - median via radix-16 select run entirely on the gpsimd/pool engine
    (partition_broadcast / partition_all_reduce), overlapped with fwd/inv
    compute of other samples; batches (0,1), (2), (3).
  - pointwise spectral stage via broadcast-AP big tensor ops (CRECIM
    interleaved dup layout built per 8-group fifth, products in place).
  - energy = square(xb bf16) on ACT + grouped tensor_reduce on DVE
    (replaces 41 ACT square+accum ops per sample).
  - PSUM-pair evictions ([128,1024] tiles, 2 chunks/4 groups per evict).
  - corner-turn DMAs spread across sync/scalar/gpsimd queues.
  - output written bf16 (within rel-err budget), 4 big output DMAs.
  - pipeline order f0 f1 M01 f2 M2 i0 i1 f3 M3 i2 i3 keeps PE busy and
    only ~2.5 xb spectra alive (SBUF).
"""
import numpy as np
import ml_dtypes

B, N, C = 32, 4097, 256
F = N // 2 + 1
BL = B // 8
NSQ = np.sqrt(np.float64(N))
FW = 17 * C  # 4352
GW = 40 * C  # 10240

# stage1/stageB column chunking: halves (2304 | 2048), chunks per half
CHUNKS = [(0, 512), (512, 512), (1024, 512), (1536, 512), (2048, 256),
          (2304, 512), (2816, 512), (3328, 512), (3840, 512)]
# eviction pairs (indices into CHUNKS): [(0,1),(2,3),(4,),(5,6),(7,8)]
EPAIRS = [(0, 1), (2, 3), (4,), (5, 6), (7, 8)]
MED_BITS = [26, 21, 16, 11]


def _build_consts():
    n1 = np.arange(241)
    k1 = np.arange(121)
    n2 = np.arange(17)
    k2 = np.arange(17)

    ang = 2 * np.pi * np.outer(n1, k1) / 241.0
    A1 = np.zeros((241, 256), np.float64)
    A1[:, 0:121] = np.cos(ang)
    A1[:, 128:248] = -np.sin(ang[:, 1:121])
    A1 /= NSQ

    def cls_mat(c):
        kk = c + 241 * k2
        th = -2 * np.pi * np.outer(n2, kk) / N
        Cm, Sm = np.cos(th), np.sin(th)
        M = np.zeros((34, 34))
        M[0:17, 0:17] = Cm
        M[17:34, 0:17] = -Sm
        M[0:17, 17:34] = Sm
        M[17:34, 17:34] = Cm
        return M

    sgn = np.ones((40, 51), np.float64)
    binm = np.zeros((42, 51), np.int64)
    for g in range(40):
        rg = g + (1 if g >= 32 else 0)
        for i in range(3):
            c = 3 * g + 1 + i
            for q in range(17):
                k = c + 241 * q
                binm[rg, 17 * i + q] = k if k <= 2048 else N - k
                if k > 2048:
                    sgn[g, 17 * i + q] = -1.0
    for q in range(9):
        binm[32, q] = 241 * q

    A2_all = np.zeros((102, 40, 116), np.float64)
    for g in range(40):
        for i in range(3):
            c = 3 * g + 1 + i
            M = cls_mat(c)
            A2_all[17 * i:17 * i + 17, g, 17 * i:17 * i + 17] = M[0:17, 0:17]
            A2_all[51 + 17 * i:51 + 17 * i + 17, g, 17 * i:17 * i + 17] = M[17:34, 0:17]
            A2_all[17 * i:17 * i + 17, g, 64 + 17 * i:64 + 17 * i + 17] = M[0:17, 17:34]
            A2_all[51 + 17 * i:51 + 17 * i + 17, g, 64 + 17 * i:64 + 17 * i + 17] = M[17:34, 17:34]
    for g in range(40):
        A2_all[:, g, 64:115] *= sgn[g][None, :]
    A2f = A2_all.reshape(102, 40 * 116)

    kk0 = 241 * np.arange(9)
    th0 = -2 * np.pi * np.outer(n2, kk0) / N
    A2_0 = np.zeros((17, 42), np.float64)
    A2_0[:, 0:9] = np.cos(th0)
    A2_0[:, 32:41] = np.sin(th0)

    def cls_inv(c):
        kk = c + 241 * k2
        th = +2 * np.pi * np.outer(n2, kk) / N
        Cm, Sm = np.cos(th), np.sin(th)
        M = np.zeros((34, 34))
        M[0:17, 0:17] = Cm.T
        M[17:34, 0:17] = -Sm.T
        M[0:17, 17:34] = Sm.T
        M[17:34, 17:34] = Cm.T
        return M / NSQ

    Ainv_all = np.zeros((116, 40, 116), np.float64)
    for g in range(40):
        for i in range(3):
            c = 3 * g + 1 + i
            M = cls_inv(c)
            Ainv_all[17 * i:17 * i + 17, g, 17 * i:17 * i + 17] = M[0:17, 0:17]
            Ainv_all[64 + 17 * i:64 + 17 * i + 17, g, 17 * i:17 * i + 17] = M[17:34, 0:17]
            Ainv_all[17 * i:17 * i + 17, g, 64 + 17 * i:64 + 17 * i + 17] = M[0:17, 17:34]
            Ainv_all[64 + 17 * i:64 + 17 * i + 17, g, 64 + 17 * i:64 + 17 * i + 17] = M[17:34, 17:34]

    AINV2 = Ainv_all.copy()
    AINVSWP = np.zeros_like(Ainv_all)
    for g in range(40):
        AINV2[64:115, g, :] = Ainv_all[64:115, g, :] * sgn[g][:, None]
        AINVSWP[0:51, g, :] = AINV2[64:115, g, :]
        AINVSWP[64:115, g, :] = -Ainv_all[0:51, g, :]
    AINV2 = AINV2.reshape(116, 40 * 116)
    AINVSWP = AINVSWP.reshape(116, 40 * 116)

    th = 2 * np.pi * np.outer(np.arange(9), n2) / 17.0
    Ainv0 = np.zeros((42, 18), np.float64)
    Ainv0[0, 0:17] = 1.0
    Ainv0[1:9, 0:17] = 2 * np.cos(th[1:9])
    Ainv0[33:41, 0:17] = -2 * np.sin(th[1:9])
    Ainv0 /= NSQ
    AINV0SWP = np.zeros_like(Ainv0)
    AINV0SWP[0:9, :] = Ainv0[32:41, :]
    AINV0SWP[32:41, :] = -Ainv0[0:9, :]

    ang2 = 2 * np.pi * np.outer(k1, n1) / 241.0
    ck = np.where(k1 == 0, 1.0, 2.0)
    cosr = ck[:, None] * np.cos(ang2)
    sinr = -2.0 * np.sin(ang2[1:121])
    B1 = np.zeros((241, 256), np.float64)
    B1[0:121, 0:128] = cosr[:, 0:128]
    B1[0:121, 128:241] = cosr[:, 128:241]
    B1[121:128, 0:128] = sinr[0:7, 0:128]
    B1[121:128, 128:241] = sinr[0:7, 128:241]
    B1[128:241, 0:128] = sinr[7:120, 0:128]
    B1[128:241, 128:241] = sinr[7:120, 128:241]

    # class-interleaved permutations (same as baseline)
    A1P = np.zeros((241, 256), np.float64)
    A1P[:, 0] = A1[:, 0]
    for c in range(1, 64):
        A1P[:, 2 * c - 1] = A1[:, c]
        A1P[:, 2 * c] = A1[:, 127 + c]
    for c in range(64, 121):
        A1P[:, 128 + 2 * (c - 64)] = A1[:, c]
        A1P[:, 128 + 2 * (c - 64) + 1] = A1[:, 127 + c]

    oldidx = np.zeros(102, np.int64)
    for i in range(3):
        for q in range(17):
            oldidx[34 * i + q] = 17 * i + q
            oldidx[34 * i + 17 + q] = 51 + 17 * i + q
    A2P = A2f[oldidx]

    mold = np.zeros(102, np.int64)
    for i in range(3):
        for q in range(17):
            mold[34 * i + q] = 17 * i + q
            mold[34 * i + 17 + q] = 64 + 17 * i + q
    AINVP = AINV2.reshape(116, 40, 116)[:, :, mold].reshape(116, 40 * 102)
    AINVSP = AINVSWP.reshape(116, 40, 116)[:, :, mold].reshape(116, 40 * 102)

    B1P = np.zeros((241, 256), np.float64)
    B1P[0] = B1[0]
    for c in range(1, 64):
        B1P[2 * c - 1] = B1[c]
        B1P[2 * c] = B1[120 + c]
    for c in range(64, 121):
        B1P[127 + 2 * (c - 64)] = B1[c]
        B1P[127 + 2 * (c - 64) + 1] = B1[120 + c]

    # radix-32 candidate increments: KB[0, 32*r + k] = (k+1) << MED_BITS[r]
    KB = np.zeros((1, 32 * len(MED_BITS)), np.uint32)
    for r, b in enumerate(MED_BITS):
        for k in range(31):
            KB[0, 32 * r + k] = np.uint32((k + 1) << b)
        KB[0, 32 * r + 31] = np.uint32(31 << b)

    bf = ml_dtypes.bfloat16
    f16 = np.float16
    return {
        "A1": A1P.astype(f16), "A2": A2P.astype(f16),
        "A20": A2_0.astype(f16), "AINV": AINVP.astype(bf),
        "AINVS": AINVSP.astype(bf),
        "AINV0": Ainv0.astype(bf), "AINV0S": AINV0SWP.astype(bf),
        "B1": B1P.astype(bf), "KB": KB,
    }, binm


_CONSTS, _BINM = _build_consts()
_NC_CACHE = {}


def _build_nc():
    if "nc" in _NC_CACHE:
        return _NC_CACHE["nc"]
    from contextlib import ExitStack
    from concourse import bacc, tile, mybir
    from concourse import bass_isa
    f32 = mybir.dt.float32
    f16 = mybir.dt.float16
    bf16 = mybir.dt.bfloat16
    u32 = mybir.dt.uint32
    Alu = mybir.AluOpType
    Act = mybir.ActivationFunctionType
    RedOp = bass_isa.ReduceOp

    nc = bacc.Bacc("TRN2", target_bir_lowering=False, debug=False, num_devices=8)
    x_t = nc.dram_tensor("x", [BL, N, C], f32, kind="ExternalInput")
    thr_t = nc.dram_tensor("thrp", [42, 51], f32, kind="ExternalInput")
    whd_t = nc.dram_tensor("WHD", [116, 512], bf16, kind="ExternalInput")
    wd_t = nc.dram_tensor("WD", [116, 512], bf16, kind="ExternalInput")
    a1_t = nc.dram_tensor("A1", [241, 256], f16, kind="ExternalInput")
    a2_t = nc.dram_tensor("A2", [102, 40 * 116], f16, kind="ExternalInput")
    a20_t = nc.dram_tensor("A20", [17, 42], f16, kind="ExternalInput")
    ainv_t = nc.dram_tensor("AINV", [116, 40 * 102], bf16, kind="ExternalInput")
    ainvs_t = nc.dram_tensor("AINVS", [116, 40 * 102], bf16, kind="ExternalInput")
    ainv0_t = nc.dram_tensor("AINV0", [42, 18], bf16, kind="ExternalInput")
    ainv0s_t = nc.dram_tensor("AINV0S", [42, 18], bf16, kind="ExternalInput")
    b1_t = nc.dram_tensor("B1", [241, 256], bf16, kind="ExternalInput")
    kb_t = nc.dram_tensor("KB", [1, 32 * len(MED_BITS)], u32, kind="ExternalInput")
    out_t = nc.dram_tensor("out", [BL, N, C], bf16, kind="ExternalOutput")

    with tile.TileContext(nc) as tc, ExitStack() as ES:
        cpool = ES.enter_context(tc.tile_pool(name="consts", bufs=1))
        xin_p = ES.enter_context(tc.tile_pool(name="xin", bufs=1))
        t_p = ES.enter_context(tc.tile_pool(name="tst", bufs=2))
        mt_p = ES.enter_context(tc.tile_pool(name="mt", bufs=6))
        xb_p = ES.enter_context(tc.tile_pool(name="xb", bufs=3))
        sq_p = ES.enter_context(tc.tile_pool(name="sq", bufs=1))
        e_p = ES.enter_context(tc.tile_pool(name="energy", bufs=2))
        eb_p = ES.enter_context(tc.tile_pool(name="ebins", bufs=1))
        x0_p = ES.enter_context(tc.tile_pool(name="x0f", bufs=1))
        med_p = ES.enter_context(tc.tile_pool(name="med", bufs=1))
        msk_p = ES.enter_context(tc.tile_pool(name="mask", bufs=2))
        cc_p = ES.enter_context(tc.tile_pool(name="crecim", bufs=2))
        ssb_p = ES.enter_context(tc.tile_pool(name="ssb", bufs=3))
        osb_p = ES.enter_context(tc.tile_pool(name="osb", bufs=1))
        y0_p = ES.enter_context(tc.tile_pool(name="cls0", bufs=1))
        pA = ES.enter_context(tc.tile_pool(name="pA", bufs=2, space="PSUM"))
        pB = ES.enter_context(tc.tile_pool(name="pB", bufs=2, space="PSUM"))

        # ---------------- constants ----------------
        a1k0 = cpool.tile([128, 256], f16)
        a1k1 = cpool.tile([113, 256], f16)
        nc.sync.dma_start(out=a1k0, in_=a1_t.ap()[0:128, :])
        nc.sync.dma_start(out=a1k1, in_=a1_t.ap()[128:241, :])
        a2_sb = cpool.tile([102, 40 * 116], f16)
        nc.scalar.dma_start(out=a2_sb, in_=a2_t.ap())
        a20_sb = cpool.tile([17, 42], f16)
        nc.sync.dma_start(out=a20_sb, in_=a20_t.ap())
        ainv_sb = cpool.tile([116, 40 * 102], bf16)
        nc.sync.dma_start(out=ainv_sb, in_=ainv_t.ap())
        ainvs_sb = cpool.tile([116, 40 * 102], bf16)
        nc.scalar.dma_start(out=ainvs_sb, in_=ainvs_t.ap())
        ainv0_sb = cpool.tile([42, 18], bf16)
        nc.sync.dma_start(out=ainv0_sb, in_=ainv0_t.ap())
        ainv0s_sb = cpool.tile([42, 18], bf16)
        nc.sync.dma_start(out=ainv0s_sb, in_=ainv0s_t.ap())
        b1k0 = cpool.tile([127, 256], bf16)
        b1k1 = cpool.tile([114, 256], bf16)
        nc.sync.dma_start(out=b1k0, in_=b1_t.ap()[0:127, :])
        nc.sync.dma_start(out=b1k1, in_=b1_t.ap()[127:241, :])
        whd = cpool.tile([116, 512], bf16)
        wd = cpool.tile([116, 512], bf16)
        nc.sync.dma_start(out=whd, in_=whd_t.ap())
        nc.sync.dma_start(out=wd, in_=wd_t.ap())
        kb = cpool.tile([1, 32 * len(MED_BITS)], u32)
        nc.sync.dma_start(out=kb, in_=kb_t.ap())
        thrp = cpool.tile([42, 51], f32)
        nc.sync.dma_start(out=thrp, in_=thr_t.ap())

        whd_v = whd.rearrange("p (h c) -> p h c", h=2, c=256)
        wd_v = wd.rearrange("p (h c) -> p h c", h=2, c=256)

        all_ebins = [None] * BL
        all_xb = [None] * BL
        all_x0f = [None] * BL
        den42 = [med_p.tile([42, 1], f32, tag=f"den{s}", name=f"den42_{s}")
                 for s in range(BL)]

        ev_cnt = [0]

        def evict(dst, src):
            # rotate PSUM evictions between DVE and ACT
            eng = (nc.vector, nc.scalar)[ev_cnt[0] % 2]
            ev_cnt[0] += 1
            if eng is nc.vector:
                eng.tensor_copy(out=dst, in_=src)
            else:
                eng.copy(out=dst, in_=src)

        ct_cnt = [0]

        def ct_eng(pool_ok):
            engs = (nc.sync, nc.scalar, nc.gpsimd)
            e = engs[ct_cnt[0] % 3]
            ct_cnt[0] += 1
            return e

        # ================= forward =================
        xin_cur = {}

        def loads(s):
            xv = x_t.ap().rearrange("s (a b) c -> s a b c", a=241, b=17)
            xin0a = xin_p.tile([128, 2304], f16, tag="x0a")
            xin1a = xin_p.tile([113, 2304], f16, tag="x1a")
            xin0b = xin_p.tile([128, 2048], f16, tag="x0b")
            xin1b = xin_p.tile([113, 2048], f16, tag="x1b")
            nc.gpsimd.dma_start(out=xin0a, in_=xv[s:s + 1, 0:128, 0:9])
            nc.gpsimd.dma_start(out=xin1a, in_=xv[s:s + 1, 128:241, 0:9])
            nc.gpsimd.dma_start(out=xin0b, in_=xv[s:s + 1, 0:128, 9:17])
            nc.gpsimd.dma_start(out=xin1b, in_=xv[s:s + 1, 128:241, 9:17])
            xin_cur[s] = (xin0a, xin1a, xin0b, xin1b)

        def fwd(s, pool_ok):
            xin0a, xin1a, xin0b, xin1b = xin_cur.pop(s)

            def xin_slice(ci):
                lo, w = CHUNKS[ci]
                if lo < 2304:
                    return (xin0a[:, lo:lo + w], xin1a[:, lo:lo + w])
                lo -= 2304
                return (xin0b[:, lo:lo + w], xin1b[:, lo:lo + w])

            t0 = t_p.tile([128, FW], f16, tag="t0")
            t1 = t_p.tile([114, FW], f16, tag="t1")
            for mt in range(2):
                dst, rows = (t0, 128) if mt == 0 else (t1, 114)
                for pair in EPAIRS:
                    ps = pA.tile([128, 1024], f32, tag="pa")
                    off = 0
                    for ci in pair:
                        lo, w = CHUNKS[ci]
                        x0s, x1s = xin_slice(ci)
                        nc.tensor.matmul(ps[:, off:off + w],
                                         a1k0[:, 128 * mt:128 * mt + 128],
                                         x0s, start=True, stop=False)
                        nc.tensor.matmul(ps[:, off:off + w],
                                         a1k1[:, 128 * mt:128 * mt + 128],
                                         x1s, start=False, stop=True)
                        off += w
                    lo0 = CHUNKS[pair[0]][0]
                    evict(dst[0:rows, lo0:lo0 + off], ps[0:rows, 0:off])

            # corner turn into per-group mt tiles, then stage2
            xb = xb_p.tile([116, GW], bf16, tag="xb")
            all_xb[s] = xb
            e2 = e_p.tile([128, 64], f32, tag="e2")
            nc.vector.memset(e2, 0.0)
            nc.vector.memset(e2[0:128, 41:42], 5.0e29)

            for gq in range(10):  # 4 groups per psum tile
                xps = pB.tile([128, 1024], f32, tag="pb")
                for k in range(4):
                    g = 4 * gq + k
                    mt_g = mt_p.tile([102, 256], f16, tag="m")
                    ctsrc = (t0[6 * g + 1:6 * g + 7, :] if g <= 20 else
                             t1[6 * (g - 21):6 * (g - 21) + 6, :]).rearrange(
                                 "i (q c) -> i q c", q=17, c=256)
                    ct_eng(pool_ok).dma_start(out=mt_g, in_=ctsrc)
                    nc.tensor.matmul(xps[0:116, 256 * k:256 * k + 256],
                                     a2_sb[:, 116 * g:116 * g + 116],
                                     mt_g, start=True, stop=True)
                evict(xb[:, 1024 * gq:1024 * gq + 1024], xps[0:116, :])

            # class 0
            m0 = mt_p.tile([17, 256], f16, tag="mc0")
            nc.sync.dma_start(
                out=m0,
                in_=t0[0:1, :].rearrange("i (q c) -> i q c", q=17, c=256))
            x0ps = pB.tile([128, 1024], f32, tag="pb")
            nc.tensor.matmul(x0ps[0:42, 0:256], a20_sb, m0,
                             start=True, stop=True)
            x0f = x0_p.tile([42, 256], bf16, tag=f"x0f_{s}")
            nc.scalar.copy(out=x0f, in_=x0ps[0:42, 0:256])
            all_x0f[s] = x0f

            # energy: square xb (ACT) then grouped reduce (DVE)
            xbv = xb.rearrange("p (g c) -> p g c", g=40, c=256)
            for f5 in range(5):
                sq = sq_p.tile([116, 2048], bf16, tag="sq")
                sqv = sq.rearrange("p (g c) -> p g c", g=8, c=256)
                nc.scalar.activation(out=sq, in_=xb[:, 2048 * f5:2048 * f5 + 2048],
                                     func=Act.Square)
                rg0 = 8 * f5 + (1 if f5 == 4 else 0)
                nc.vector.tensor_reduce(
                    out=e2[0:116, rg0:rg0 + 8], in_=sqv,
                    axis=mybir.AxisListType.X, op=Alu.add)
            sq0 = sq_p.tile([42, 256], bf16, tag="sq0")
            nc.scalar.activation(out=sq0, in_=x0f, func=Act.Square)
            nc.vector.tensor_reduce(
                out=e2[0:42, 32:33], in_=sq0[:, None, :],
                axis=mybir.AxisListType.X, op=Alu.add)

            e2T = e_p.tile([64, 128], f32, tag="e2T")
            for a in range(4):
                for bb in range(2):
                    nc.vector.transpose(
                        out=e2T[32 * bb:32 * bb + 32, 32 * a:32 * a + 32],
                        in_=e2[32 * a:32 * a + 32, 32 * bb:32 * bb + 32])
            ebins = eb_p.tile([42, 51], f32, tag=f"eb{s}")
            nc.vector.tensor_add(ebins[0:42, 0:51], e2T[0:42, 0:51],
                                 e2T[0:42, 64:115])
            nc.vector.memset(ebins[32:33, 9:51], 1.0e30)
            nc.vector.tensor_add(ebins[32:33, 0:9], ebins[32:33, 0:9],
                                 e2T[32:33, 32:41])
            all_ebins[s] = ebins

        # ========== median (radix-32 select, DVE + pool broadcast) ==========
        def median(s):
            e_rep = med_p.tile([32, 2142], f32, tag="erep")
            cjunk = med_p.tile([32, 2142], bf16, tag="cjunk")
            candp = med_p.tile([32, 32], f32, tag="candp")
            candT = med_p.tile([32, 32], f32, tag="candT")
            cntp = med_p.tile([32, 32], f32, tag="cntp")
            cntT = med_p.tile([32, 32], f32, tag="cntT")
            P = med_p.tile([1, 1], u32, tag="P")
            dd = med_p.tile([1, 1], f32, tag="dd")
            stepf = med_p.tile([1, 1], f32, tag="stepf")
            stepu = med_p.tile([1, 1], u32, tag="stepu")
            nc.sync.dma_start(out=e_rep[0:1, :], in_=all_ebins[s])
            nc.gpsimd.partition_broadcast(out_ap=e_rep, in_ap=e_rep[0:1, :])
            nc.vector.memset(P, 0)
            nc.vector.memset(candp, 0.0)
            for r, b in enumerate(MED_BITS):
                nc.vector.tensor_tensor(
                    out=candp[0:1, :].bitcast(u32),
                    in0=kb[:, 32 * r:32 * r + 32],
                    in1=P.to_broadcast((1, 32)), op=Alu.add)
                nc.vector.transpose(out=candT, in_=candp)
                nc.vector.tensor_scalar(
                    out=cjunk, in0=e_rep, scalar1=candT[0:32, 0:1],
                    scalar2=0.0, op0=Alu.is_lt, op1=Alu.add,
                    accum_out=cntp[0:32, 0:1])
                nc.vector.transpose(out=cntT, in_=cntp)
                nc.vector.tensor_scalar(
                    out=cntT[0:1, 0:31], in0=cntT[0:1, 0:31], scalar1=1024.5,
                    scalar2=0.0, op0=Alu.is_lt, op1=Alu.add, accum_out=dd)
                nc.vector.tensor_scalar(out=stepf, in0=dd,
                                        scalar1=float(1 << b), scalar2=None,
                                        op0=Alu.mult)
                nc.vector.tensor_copy(out=stepu, in_=stepf)
                nc.vector.tensor_tensor(out=P, in0=P, in1=stepu, op=Alu.add)
            nc.gpsimd.partition_broadcast(out_ap=den42[s], in_ap=P.bitcast(f32))
            nc.vector.tensor_scalar(out=den42[s], in0=den42[s],
                                    scalar1=1.0e-6, scalar2=None, op0=Alu.add)

        # ================= inverse =================
        def inv(s, pool_ok):
            ebins = all_ebins[s]
            xb = all_xb[s]
            x0f = all_x0f[s]
            ths = msk_p.tile([42, 51], f32, tag="ths")
            nc.vector.tensor_scalar(out=ths, in0=thrp, scalar1=den42[s],
                                    scalar2=None, op0=Alu.mult)
            hardP = msk_p.tile([64, 64], f32, tag="hardP")
            nc.vector.memset(hardP, 0.0)
            nc.vector.tensor_tensor(out=hardP[0:42, 0:51], in0=ebins,
                                    in1=ths, op=Alu.is_gt)
            mTf = msk_p.tile([64, 64], f32, tag="mTf")
            for a in range(2):
                for bb in range(2):
                    nc.vector.transpose(
                        out=mTf[32 * bb:32 * bb + 32, 32 * a:32 * a + 32],
                        in_=hardP[32 * a:32 * a + 32, 32 * bb:32 * bb + 32])
            # dense group-mask [116, 41]: cols 0..39 = groups, col 40 unused
            mT2b = msk_p.tile([116, 41], bf16, tag="mT2b")
            nc.vector.memset(mT2b, 0.0)
            nc.vector.tensor_copy(out=mT2b[0:51, 0:32], in_=mTf[0:51, 0:32])
            nc.vector.tensor_copy(out=mT2b[0:51, 32:40], in_=mTf[0:51, 33:41])
            nc.vector.tensor_copy(out=mT2b[64:115, 0:32], in_=mTf[0:51, 0:32])
            nc.vector.tensor_copy(out=mT2b[64:115, 32:40], in_=mTf[0:51, 33:41])
            m0c = msk_p.tile([42, 1], bf16, tag="m0c")
            nc.vector.memset(m0c, 0.0)
            nc.vector.tensor_copy(out=m0c[0:9, 0:1], in_=mTf[0:9, 32:33])
            nc.vector.tensor_copy(out=m0c[32:41, 0:1], in_=mTf[0:9, 32:33])

            st0f = t_p.tile([128, FW], f16, tag="t0", name="st0f")
            st1f = t_p.tile([114, FW], f16, tag="t1", name="st1f")
            st0 = st0f.bitcast(bf16)
            st1 = st1f.bitcast(bf16)
            xbv = xb.rearrange("p (g c) -> p g c", g=40, c=256)

            for f5 in range(5):  # 8 groups per fifth
                g0 = 8 * f5
                cc = cc_p.tile([116, 8 * 512], bf16, tag="cc")
                ccv = cc.rearrange("p (h g c) -> p h g c", h=2, g=8, c=256)
                maskbc = mT2b[:, None, g0:g0 + 8, None].broadcast_to(
                    (116, 2, 8, 256))
                whbc = whd_v[:, :, None, :].broadcast_to((116, 2, 8, 256))
                wdbc = wd_v[:, :, None, :].broadcast_to((116, 2, 8, 256))
                nc.vector.tensor_tensor(out=ccv, in0=maskbc, in1=whbc,
                                        op=Alu.mult)
                nc.vector.tensor_tensor(out=ccv, in0=ccv, in1=wdbc,
                                        op=Alu.add)
                nc.vector.tensor_tensor(out=cc[:, 0:2048],
                                        in0=xb[:, 2048 * f5:2048 * f5 + 2048],
                                        in1=cc[:, 0:2048], op=Alu.mult)
                nc.vector.tensor_tensor(out=cc[:, 2048:4096],
                                        in0=xb[:, 2048 * f5:2048 * f5 + 2048],
                                        in1=cc[:, 2048:4096], op=Alu.mult)
                for hq in range(2):
                    sps = pA.tile([128, 1024], f32, tag="pa")
                    for k in range(4):
                        g = g0 + 4 * hq + k
                        nc.tensor.matmul(
                            sps[0:102, 256 * k:256 * k + 256],
                            ainv_sb[:, 102 * g:102 * g + 102],
                            ccv[:, 0, 4 * hq + k, :], start=True, stop=False)
                        nc.tensor.matmul(
                            sps[0:102, 256 * k:256 * k + 256],
                            ainvs_sb[:, 102 * g:102 * g + 102],
                            ccv[:, 1, 4 * hq + k, :], start=False, stop=True)
                    ssb = ssb_p.tile([102, 1024], bf16, tag="ssb")
                    evict(ssb, sps[0:102, :])
                    for k in range(4):
                        g = g0 + 4 * hq + k
                        ctdst = (st0[6 * g + 1:6 * g + 7, :] if g <= 20 else
                                 st1[6 * (g - 21):6 * (g - 21) + 6, :]
                                 ).rearrange("i (q c) -> i q c", q=17, c=256)
                        ct_eng(pool_ok).dma_start(
                            out=ctdst, in_=ssb[:, 256 * k:256 * k + 256])

            # class 0
            cre0 = y0_p.tile([42, 256], bf16, tag="cre0")
            cim0 = y0_p.tile([42, 256], bf16, tag="cim0")
            nc.vector.scalar_tensor_tensor(
                out=cre0, in0=whd[0:42, 0:256], scalar=m0c,
                in1=wd[0:42, 0:256], op0=Alu.mult, op1=Alu.add)
            nc.vector.scalar_tensor_tensor(
                out=cim0, in0=whd[0:42, 256:512], scalar=m0c,
                in1=wd[0:42, 256:512], op0=Alu.mult, op1=Alu.add)
            nc.vector.tensor_tensor(out=cre0, in0=x0f, in1=cre0, op=Alu.mult)
            nc.vector.tensor_tensor(out=cim0, in0=x0f, in1=cim0, op=Alu.mult)
            s0ps = pA.tile([128, 1024], f32, tag="pa")
            nc.tensor.matmul(s0ps[0:18, 0:256], ainv0_sb, cre0,
                             start=True, stop=False)
            nc.tensor.matmul(s0ps[0:18, 0:256], ainv0s_sb, cim0,
                             start=False, stop=True)
            s0sb = ssb_p.tile([18, 256], bf16, tag="sc0")
            nc.scalar.copy(out=s0sb, in_=s0ps[0:18, 0:256])
            nc.sync.dma_start(
                out=st0[0:1, :].rearrange("i (q c) -> i q c", q=17, c=256),
                in_=s0sb[0:17, :])

            # stage B, column-half major; output DMA per (colhalf, mt)
            ov = out_t.ap().rearrange("s (a b) c -> s a b c", a=241, b=17)
            for ch in range(2):
                pr = EPAIRS[0:3] if ch == 0 else EPAIRS[3:5]
                wtot = 2304 if ch == 0 else 2048
                lo_h = 0 if ch == 0 else 2304
                osb0 = osb_p.tile([128, 2304], bf16, tag="osb0")
                osb1 = osb_p.tile([113, 2304], bf16, tag="osb1")
                for mt in range(2):
                    dst, rows = (osb0, 128) if mt == 0 else (osb1, 113)
                    for pair in pr:
                        ps = pB.tile([128, 1024], f32, tag="pb")
                        off = 0
                        for ci in pair:
                            lo, w = CHUNKS[ci]
                            nc.tensor.matmul(ps[:, off:off + w],
                                             b1k0[:, 128 * mt:128 * mt + 128],
                                             st0[0:127, lo:lo + w],
                                             start=True, stop=False)
                            nc.tensor.matmul(ps[:, off:off + w],
                                             b1k1[:, 128 * mt:128 * mt + 128],
                                             st1[:, lo:lo + w],
                                             start=False, stop=True)
                            off += w
                        lo0 = CHUNKS[pair[0]][0] - lo_h
                        evict(dst[0:rows, lo0:lo0 + off], ps[0:rows, 0:off])
                    b_lo, b_n = (0, 9) if ch == 0 else (9, 8)
                    oeng = (nc.sync, nc.scalar)[(ch + mt) % 2]
                    oeng.dma_start(
                        out=ov[s:s + 1, 128 * mt:128 * mt + rows,
                               b_lo:b_lo + b_n, :],
                        in_=dst[0:rows, 0:wtot].rearrange(
                            "p (q c) -> p q c", q=b_n, c=256))

        # ================= pipeline =================
        loads(0)
        fwd(0, True)
        loads(1)
        fwd(1, True)
        loads(2)
        median(0)
        median(1)
        fwd(2, False)
        loads(3)
        median(2)
        inv(0, False)
        inv(1, False)
        fwd(3, False)
        median(3)
        inv(2, True)
        inv(3, True)

    nc.compile()
    _NC_CACHE["nc"] = nc
    return nc


def _make_in_maps(x_in, complex_weight, complex_weight_high, threshold_param):
    bf = ml_dtypes.bfloat16
    thrp = np.asarray(threshold_param, np.float32)[_BINM.reshape(-1)]
    thrp = np.ascontiguousarray(thrp.reshape(42, 51))
    cw = np.asarray(complex_weight, np.float32)
    cwh = np.asarray(complex_weight_high, np.float32)
    whd = np.zeros((116, 512), np.float32)
    whd[:, 0:256] = cwh[:, 0]
    whd[:, 256:512] = cwh[:, 1]
    wdm = np.zeros((116, 512), np.float32)
    wdm[:, 0:256] = cw[:, 0]
    wdm[:, 256:512] = cw[:, 1]
    whd = whd.astype(bf)
    wdm = wdm.astype(bf)

    x_in = np.ascontiguousarray(np.asarray(x_in, np.float32))
    in_maps = []
    for core in range(8):
        m = {"x": x_in[BL * core:BL * core + BL],
             "thrp": thrp, "WHD": whd, "WD": wdm}
        m.update(_CONSTS)
        in_maps.append(m)
    return in_maps


def kernel(x_in, complex_weight, complex_weight_high, threshold_param):
    from concourse.bass_utils import run_bass_kernel_spmd
    nc = _build_nc()
    in_maps = _make_in_maps(x_in, complex_weight, complex_weight_high,
                            threshold_param)
    res = run_bass_kernel_spmd(nc, in_maps, core_ids=list(range(8)))
    out = np.concatenate([np.asarray(res.results[i]["out"], np.float32)
                          for i in range(8)], axis=0)
    return out


# revision 4
# speedup vs baseline: 1.0238x; 1.0238x over previous
"""Adaptive Spectral Block on 8 TRN2 NeuronCores (data-parallel over batch).

v2 of the matmul-FFT kernel (N = 4097 = 241*17 Cooley-Tukey):
  - median via radix-16 select run entirely on the gpsimd/pool engine
    (partition_broadcast / partition_all_reduce), overlapped with fwd/inv
    compute of other samples; batches (0,1), (2), (3).
  - pointwise spectral stage via broadcast-AP big tensor ops (CRECIM
    interleaved dup layout built per 8-group fifth, products in place).
  - energy = square(xb bf16) on ACT + grouped tensor_reduce on DVE
    (replaces 41 ACT square+accum ops per sample).
  - PSUM-pair evictions ([128,1024] tiles, 2 chunks/4 groups per evict).
  - corner-turn DMAs spread across sync/scalar/gpsimd queues.
  - output written bf16 (within rel-err budget), 4 big output DMAs.
  - pipeline order f0 f1 M01 f2 M2 i0 i1 f3 M3 i2 i3 keeps PE busy and
    only ~2.5 xb spectra alive (SBUF).
"""
import numpy as np
import ml_dtypes

B, N, C = 32, 4097, 256
F = N // 2 + 1
BL = B // 8
NSQ = np.sqrt(np.float64(N))
FW = 17 * C  # 4352
GW = 40 * C  # 10240

# stage1/stageB column chunking: halves (2304 | 2048), chunks per half
CHUNKS = [(0, 512), (512, 512), (1024, 512), (1536, 512), (2048, 256),
          (2304, 512), (2816, 512), (3328, 512), (3840, 512)]
# eviction pairs (indices into CHUNKS): [(0,1),(2,3),(4,),(5,6),(7,8)]
EPAIRS = [(0, 1), (2, 3), (4,), (5, 6), (7, 8)]
MED_BITS = [26, 21, 16, 11]


def _build_consts():
    n1 = np.arange(241)
    k1 = np.arange(121)
    n2 = np.arange(17)
    k2 = np.arange(17)

    ang = 2 * np.pi * np.outer(n1, k1) / 241.0
    A1 = np.zeros((241, 256), np.float64)
    A1[:, 0:121] = np.cos(ang)
    A1[:, 128:248] = -np.sin(ang[:, 1:121])
    A1 /= NSQ

    def cls_mat(c):
        kk = c + 241 * k2
        th = -2 * np.pi * np.outer(n2, kk) / N
        Cm, Sm = np.cos(th), np.sin(th)
        M = np.zeros((34, 34))
        M[0:17, 0:17] = Cm
        M[17:34, 0:17] = -Sm
        M[0:17, 17:34] = Sm
        M[17:34, 17:34] = Cm
        return M

    sgn = np.ones((40, 51), np.float64)
    binm = np.zeros((42, 51), np.int64)
    for g in range(40):
        rg = g + (1 if g >= 32 else 0)
        for i in range(3):
            c = 3 * g + 1 + i
            for q in range(17):
                k = c + 241 * q
                binm[rg, 17 * i + q] = k if k <= 2048 else N - k
                if k > 2048:
                    sgn[g, 17 * i + q] = -1.0
    for q in range(9):
        binm[32, q] = 241 * q

    A2_all = np.zeros((102, 40, 116), np.float64)
    for g in range(40):
        for i in range(3):
            c = 3 * g + 1 + i
            M = cls_mat(c)
            A2_all[17 * i:17 * i + 17, g, 17 * i:17 * i + 17] = M[0:17, 0:17]
            A2_all[51 + 17 * i:51 + 17 * i + 17, g, 17 * i:17 * i + 17] = M[17:34, 0:17]
            A2_all[17 * i:17 * i + 17, g, 64 + 17 * i:64 + 17 * i + 17] = M[0:17, 17:34]
            A2_all[51 + 17 * i:51 + 17 * i + 17, g, 64 + 17 * i:64 + 17 * i + 17] = M[17:34, 17:34]
    for g in range(40):
        A2_all[:, g, 64:115] *= sgn[g][None, :]
    A2f = A2_all.reshape(102, 40 * 116)

    kk0 = 241 * np.arange(9)
    th0 = -2 * np.pi * np.outer(n2, kk0) / N
    A2_0 = np.zeros((17, 42), np.float64)
    A2_0[:, 0:9] = np.cos(th0)
    A2_0[:, 32:41] = np.sin(th0)

    def cls_inv(c):
        kk = c + 241 * k2
        th = +2 * np.pi * np.outer(n2, kk) / N
        Cm, Sm = np.cos(th), np.sin(th)
        M = np.zeros((34, 34))
        M[0:17, 0:17] = Cm.T
        M[17:34, 0:17] = -Sm.T
        M[0:17, 17:34] = Sm.T
        M[17:34, 17:34] = Cm.T
        return M / NSQ

    Ainv_all = np.zeros((116, 40, 116), np.float64)
    for g in range(40):
        for i in range(3):
            c = 3 * g + 1 + i
            M = cls_inv(c)
            Ainv_all[17 * i:17 * i + 17, g, 17 * i:17 * i + 17] = M[0:17, 0:17]
            Ainv_all[64 + 17 * i:64 + 17 * i + 17, g, 17 * i:17 * i + 17] = M[17:34, 0:17]
            Ainv_all[17 * i:17 * i + 17, g, 64 + 17 * i:64 + 17 * i + 17] = M[0:17, 17:34]
            Ainv_all[64 + 17 * i:64 + 17 * i + 17, g, 64 + 17 * i:64 + 17 * i + 17] = M[17:34, 17:34]

    AINV2 = Ainv_all.copy()
    AINVSWP = np.zeros_like(Ainv_all)
    for g in range(40):
        AINV2[64:115, g, :] = Ainv_all[64:115, g, :] * sgn[g][:, None]
        AINVSWP[0:51, g, :] = AINV2[64:115, g, :]
        AINVSWP[64:115, g, :] = -Ainv_all[0:51, g, :]
    AINV2 = AINV2.reshape(116, 40 * 116)
    AINVSWP = AINVSWP.reshape(116, 40 * 116)

    th = 2 * np.pi * np.outer(np.arange(9), n2) / 17.0
    Ainv0 = np.zeros((42, 18), np.float64)
    Ainv0[0, 0:17] = 1.0
    Ainv0[1:9, 0:17] = 2 * np.cos(th[1:9])
    Ainv0[33:41, 0:17] = -2 * np.sin(th[1:9])
    Ainv0 /= NSQ
    AINV0SWP = np.zeros_like(Ainv0)
    AINV0SWP[0:9, :] = Ainv0[32:41, :]
    AINV0SWP[32:41, :] = -Ainv0[0:9, :]

    ang2 = 2 * np.pi * np.outer(k1, n1) / 241.0
    ck = np.where(k1 == 0, 1.0, 2.0)
    cosr = ck[:, None] * np.cos(ang2)
    sinr = -2.0 * np.sin(ang2[1:121])
    B1 = np.zeros((241, 256), np.float64)
    B1[0:121, 0:128] = cosr[:, 0:128]
    B1[0:121, 128:241] = cosr[:, 128:241]
    B1[121:128, 0:128] = sinr[0:7, 0:128]
    B1[121:128, 128:241] = sinr[0:7, 128:241]
    B1[128:241, 0:128] = sinr[7:120, 0:128]
    B1[128:241, 128:241] = sinr[7:120, 128:241]

    # class-interleaved permutations (same as baseline)
    A1P = np.zeros((241, 256), np.float64)
    A1P[:, 0] = A1[:, 0]
    for c in range(1, 64):
        A1P[:, 2 * c - 1] = A1[:, c]
        A1P[:, 2 * c] = A1[:, 127 + c]
    for c in range(64, 121):
        A1P[:, 128 + 2 * (c - 64)] = A1[:, c]
        A1P[:, 128 + 2 * (c - 64) + 1] = A1[:, 127 + c]

    oldidx = np.zeros(102, np.int64)
    for i in range(3):
        for q in range(17):
            oldidx[34 * i + q] = 17 * i + q
            oldidx[34 * i + 17 + q] = 51 + 17 * i + q
    A2P = A2f[oldidx]

    mold = np.zeros(102, np.int64)
    for i in range(3):
        for q in range(17):
            mold[34 * i + q] = 17 * i + q
            mold[34 * i + 17 + q] = 64 + 17 * i + q
    AINVP = AINV2.reshape(116, 40, 116)[:, :, mold].reshape(116, 40 * 102)
    AINVSP = AINVSWP.reshape(116, 40, 116)[:, :, mold].reshape(116, 40 * 102)

    B1P = np.zeros((241, 256), np.float64)
    B1P[0] = B1[0]
    for c in range(1, 64):
        B1P[2 * c - 1] = B1[c]
        B1P[2 * c] = B1[120 + c]
    for c in range(64, 121):
        B1P[127 + 2 * (c - 64)] = B1[c]
        B1P[127 + 2 * (c - 64) + 1] = B1[120 + c]

    # radix-32 candidate increments: KB[0, 32*r + k] = (k+1) << MED_BITS[r]
    KB = np.zeros((1, 32 * len(MED_BITS)), np.uint32)
    for r, b in enumerate(MED_BITS):
        for k in range(31):
            KB[0, 32 * r + k] = np.uint32((k + 1) << b)
        KB[0, 32 * r + 31] = np.uint32(31 << b)

    bf = ml_dtypes.bfloat16
    f16 = np.float16
    return {
        "A1": A1P.astype(f16), "A2": A2P.astype(f16),
        "A20": A2_0.astype(f16), "AINV": AINVP.astype(bf),
        "AINVS": AINVSP.astype(bf),
        "AINV0": Ainv0.astype(bf), "AINV0S": AINV0SWP.astype(bf),
        "B1": B1P.astype(bf), "KB": KB,
    }, binm


_CONSTS, _BINM = _build_consts()
_NC_CACHE = {}


def _build_nc():
    if "nc" in _NC_CACHE:
        return _NC_CACHE["nc"]
    from contextlib import ExitStack
    from concourse import bacc, tile, mybir
    from concourse import bass_isa
    f32 = mybir.dt.float32
    f16 = mybir.dt.float16
    bf16 = mybir.dt.bfloat16
    u32 = mybir.dt.uint32
    Alu = mybir.AluOpType
    Act = mybir.ActivationFunctionType
    RedOp = bass_isa.ReduceOp

    nc = bacc.Bacc("TRN2", target_bir_lowering=False, debug=False, num_devices=8)
    x_t = nc.dram_tensor("x", [BL, N, C], f32, kind="ExternalInput")
    thr_t = nc.dram_tensor("thrp", [42, 51], f32, kind="ExternalInput")
    whd_t = nc.dram_tensor("WHD", [116, 512], bf16, kind="ExternalInput")
    wd_t = nc.dram_tensor("WD", [116, 512], bf16, kind="ExternalInput")
    a1_t = nc.dram_tensor("A1", [241, 256], f16, kind="ExternalInput")
    a2_t = nc.dram_tensor("A2", [102, 40 * 116], f16, kind="ExternalInput")
    a20_t = nc.dram_tensor("A20", [17, 42], f16, kind="ExternalInput")
    ainv_t = nc.dram_tensor("AINV", [116, 40 * 102], bf16, kind="ExternalInput")
    ainvs_t = nc.dram_tensor("AINVS", [116, 40 * 102], bf16, kind="ExternalInput")
    ainv0_t = nc.dram_tensor("AINV0", [42, 18], bf16, kind="ExternalInput")
    ainv0s_t = nc.dram_tensor("AINV0S", [42, 18], bf16, kind="ExternalInput")
    b1_t = nc.dram_tensor("B1", [241, 256], bf16, kind="ExternalInput")
    kb_t = nc.dram_tensor("KB", [1, 32 * len(MED_BITS)], u32, kind="ExternalInput")
    out_t = nc.dram_tensor("out", [BL, N, C], bf16, kind="ExternalOutput")

    with tile.TileContext(nc) as tc, ExitStack() as ES:
        cpool = ES.enter_context(tc.tile_pool(name="consts", bufs=1))
        xin_p = ES.enter_context(tc.tile_pool(name="xin", bufs=1))
        t_p = ES.enter_context(tc.tile_pool(name="tst", bufs=2))
        mt_p = ES.enter_context(tc.tile_pool(name="mt", bufs=6))
        xb_p = ES.enter_context(tc.tile_pool(name="xb", bufs=3))
        sq_p = ES.enter_context(tc.tile_pool(name="sq", bufs=1))
        e_p = ES.enter_context(tc.tile_pool(name="energy", bufs=2))
        eb_p = ES.enter_context(tc.tile_pool(name="ebins", bufs=1))
        x0_p = ES.enter_context(tc.tile_pool(name="x0f", bufs=1))
        med_p = ES.enter_context(tc.tile_pool(name="med", bufs=1))
        msk_p = ES.enter_context(tc.tile_pool(name="mask", bufs=2))
        cc_p = ES.enter_context(tc.tile_pool(name="crecim", bufs=2))
        ssb_p = ES.enter_context(tc.tile_pool(name="ssb", bufs=3))
        osb_p = ES.enter_context(tc.tile_pool(name="osb", bufs=1))
        y0_p = ES.enter_context(tc.tile_pool(name="cls0", bufs=1))
        pA = ES.enter_context(tc.tile_pool(name="pA", bufs=2, space="PSUM"))
        pB = ES.enter_context(tc.tile_pool(name="pB", bufs=2, space="PSUM"))

        # ---------------- constants ----------------
        a1k0 = cpool.tile([128, 256], f16)
        a1k1 = cpool.tile([113, 256], f16)
        nc.sync.dma_start(out=a1k0, in_=a1_t.ap()[0:128, :])
        nc.sync.dma_start(out=a1k1, in_=a1_t.ap()[128:241, :])
        a2_sb = cpool.tile([102, 40 * 116], f16)
        nc.scalar.dma_start(out=a2_sb, in_=a2_t.ap())
        a20_sb = cpool.tile([17, 42], f16)
        nc.sync.dma_start(out=a20_sb, in_=a20_t.ap())
        ainv_sb = cpool.tile([116, 40 * 102], bf16)
        nc.sync.dma_start(out=ainv_sb, in_=ainv_t.ap())
        ainvs_sb = cpool.tile([116, 40 * 102], bf16)
        nc.scalar.dma_start(out=ainvs_sb, in_=ainvs_t.ap())
        ainv0_sb = cpool.tile([42, 18], bf16)
        nc.sync.dma_start(out=ainv0_sb, in_=ainv0_t.ap())
        ainv0s_sb = cpool.tile([42, 18], bf16)
        nc.sync.dma_start(out=ainv0s_sb, in_=ainv0s_t.ap())
        b1k0 = cpool.tile([127, 256], bf16)
        b1k1 = cpool.tile([114, 256], bf16)
        nc.sync.dma_start(out=b1k0, in_=b1_t.ap()[0:127, :])
        nc.sync.dma_start(out=b1k1, in_=b1_t.ap()[127:241, :])
        whd = cpool.tile([116, 512], bf16)
        wd = cpool.tile([116, 512], bf16)
        nc.sync.dma_start(out=whd, in_=whd_t.ap())
        nc.sync.dma_start(out=wd, in_=wd_t.ap())
        kb = cpool.tile([1, 32 * len(MED_BITS)], u32)
        nc.sync.dma_start(out=kb, in_=kb_t.ap())
        thrp = cpool.tile([42, 51], f32)
        nc.sync.dma_start(out=thrp, in_=thr_t.ap())

        whd_v = whd.rearrange("p (h c) -> p h c", h=2, c=256)
        wd_v = wd.rearrange("p (h c) -> p h c", h=2, c=256)

        all_ebins = [None] * BL
        all_xb = [None] * BL
        all_x0f = [None] * BL
        den42 = [med_p.tile([42, 1], f32, tag=f"den{s}", name=f"den42_{s}")
                 for s in range(BL)]

        ev_cnt = [0]

        def evict(dst, src):
            # rotate PSUM evictions between DVE and ACT
            eng = (nc.vector, nc.scalar)[ev_cnt[0] % 2]
            ev_cnt[0] += 1
            if eng is nc.vector:
                eng.tensor_copy(out=dst, in_=src)
            else:
                eng.copy(out=dst, in_=src)

        ct_cnt = [0]

        def ct_eng(pool_ok):
            engs = (nc.sync, nc.scalar, nc.gpsimd)
            e = engs[ct_cnt[0] % 3]
            ct_cnt[0] += 1
            return e

        # ================= forward =================
        xin_cur = {}

        def loads(s):
            xv = x_t.ap().rearrange("s (a b) c -> s a b c", a=241, b=17)
            xin0a = xin_p.tile([128, 2304], f16, tag="x0a")
            xin1a = xin_p.tile([113, 2304], f16, tag="x1a")
            xin0b = xin_p.tile([128, 2048], f16, tag="x0b")
            xin1b = xin_p.tile([113, 2048], f16, tag="x1b")
            nc.gpsimd.dma_start(out=xin0a, in_=xv[s:s + 1, 0:128, 0:9])
            nc.gpsimd.dma_start(out=xin1a, in_=xv[s:s + 1, 128:241, 0:9])
            nc.gpsimd.dma_start(out=xin0b, in_=xv[s:s + 1, 0:128, 9:17])
            nc.gpsimd.dma_start(out=xin1b, in_=xv[s:s + 1, 128:241, 9:17])
            xin_cur[s] = (xin0a, xin1a, xin0b, xin1b)

        def fwd(s, pool_ok):
            xin0a, xin1a, xin0b, xin1b = xin_cur.pop(s)

            def xin_slice(ci):
                lo, w = CHUNKS[ci]
                if lo < 2304:
                    return (xin0a[:, lo:lo + w], xin1a[:, lo:lo + w])
                lo -= 2304
                return (xin0b[:, lo:lo + w], xin1b[:, lo:lo + w])

            t0 = t_p.tile([128, FW], f16, tag="t0")
            t1 = t_p.tile([114, FW], f16, tag="t1")
            for mt in range(2):
                dst, rows = (t0, 128) if mt == 0 else (t1, 114)
                for pair in EPAIRS:
                    ps = pA.tile([128, 1024], f32, tag="pa")
                    off = 0
                    for ci in pair:
                        lo, w = CHUNKS[ci]
                        x0s, x1s = xin_slice(ci)
                        nc.tensor.matmul(ps[:, off:off + w],
                                         a1k0[:, 128 * mt:128 * mt + 128],
                                         x0s, start=True, stop=False)
                        nc.tensor.matmul(ps[:, off:off + w],
                                         a1k1[:, 128 * mt:128 * mt + 128],
                                         x1s, start=False, stop=True)
                        off += w
                    lo0 = CHUNKS[pair[0]][0]
                    evict(dst[0:rows, lo0:lo0 + off], ps[0:rows, 0:off])

            # corner turn into per-group mt tiles, then stage2
            xb = xb_p.tile([116, GW], bf16, tag="xb")
            all_xb[s] = xb
            e2 = e_p.tile([128, 64], f32, tag="e2")
            nc.vector.memset(e2, 0.0)
            nc.vector.memset(e2[0:128, 41:42], 5.0e29)

            for gq in range(10):  # 4 groups per psum tile
                xps = pB.tile([128, 1024], f32, tag="pb")
                for k in range(4):
                    g = 4 * gq + k
                    mt_g = mt_p.tile([102, 256], f16, tag="m")
                    ctsrc = (t0[6 * g + 1:6 * g + 7, :] if g <= 20 else
                             t1[6 * (g - 21):6 * (g - 21) + 6, :]).rearrange(
                                 "i (q c) -> i q c", q=17, c=256)
                    ct_eng(pool_ok).dma_start(out=mt_g, in_=ctsrc)
                    nc.tensor.matmul(xps[0:116, 256 * k:256 * k + 256],
                                     a2_sb[:, 116 * g:116 * g + 116],
                                     mt_g, start=True, stop=True)
                evict(xb[:, 1024 * gq:1024 * gq + 1024], xps[0:116, :])

            # class 0
            m0 = mt_p.tile([17, 256], f16, tag="mc0")
            nc.sync.dma_start(
                out=m0,
                in_=t0[0:1, :].rearrange("i (q c) -> i q c", q=17, c=256))
            x0ps = pB.tile([128, 1024], f32, tag="pb")
            nc.tensor.matmul(x0ps[0:42, 0:256], a20_sb, m0,
                             start=True, stop=True)
            x0f = x0_p.tile([42, 256], bf16, tag=f"x0f_{s}")
            nc.scalar.copy(out=x0f, in_=x0ps[0:42, 0:256])
            all_x0f[s] = x0f

            # energy: square xb (ACT) then grouped reduce (DVE)
            xbv = xb.rearrange("p (g c) -> p g c", g=40, c=256)
            for f5 in range(5):
                sq = sq_p.tile([116, 2048], bf16, tag="sq")
                sqv = sq.rearrange("p (g c) -> p g c", g=8, c=256)
                nc.scalar.activation(out=sq, in_=xb[:, 2048 * f5:2048 * f5 + 2048],
                                     func=Act.Square)
                rg0 = 8 * f5 + (1 if f5 == 4 else 0)
                nc.vector.tensor_reduce(
                    out=e2[0:116, rg0:rg0 + 8], in_=sqv,
                    axis=mybir.AxisListType.X, op=Alu.add)
            sq0 = sq_p.tile([42, 256], bf16, tag="sq0")
            nc.scalar.activation(out=sq0, in_=x0f, func=Act.Square)
            nc.vector.tensor_reduce(
                out=e2[0:42, 32:33], in_=sq0[:, None, :],
                axis=mybir.AxisListType.X, op=Alu.add)

            e2T = e_p.tile([64, 128], f32, tag="e2T")
            for a in range(4):
                for bb in range(2):
                    nc.vector.transpose(
                        out=e2T[32 * bb:32 * bb + 32, 32 * a:32 * a + 32],
                        in_=e2[32 * a:32 * a + 32, 32 * bb:32 * bb + 32])
            ebins = eb_p.tile([42, 51], f32, tag=f"eb{s}")
            nc.vector.tensor_add(ebins[0:42, 0:51], e2T[0:42, 0:51],
                                 e2T[0:42, 64:115])
            nc.vector.memset(ebins[32:33, 9:51], 1.0e30)
            nc.vector.tensor_add(ebins[32:33, 0:9], ebins[32:33, 0:9],
                                 e2T[32:33, 32:41])
            all_ebins[s] = ebins

        # ========== median (radix-32 select, DVE + pool broadcast) ==========
        def median(s):
            e_rep = med_p.tile([32, 2142], f32, tag="erep")
            cjunk = med_p.tile([32, 2142], bf16, tag="cjunk")
            candp = med_p.tile([32, 32], f32, tag="candp")
            candT = med_p.tile([32, 32], f32, tag="candT")
            cntp = med_p.tile([32, 32], f32, tag="cntp")
            cntT = med_p.tile([32, 32], f32, tag="cntT")
            P = med_p.tile([1, 1], u32, tag="P")
            dd = med_p.tile([1, 1], f32, tag="dd")
            stepf = med_p.tile([1, 1], f32, tag="stepf")
            stepu = med_p.tile([1, 1], u32, tag="stepu")
            nc.sync.dma_start(out=e_rep[0:1, :], in_=all_ebins[s])
            nc.gpsimd.partition_broadcast(out_ap=e_rep, in_ap=e_rep[0:1, :])
            nc.vector.memset(P, 0)
            nc.vector.memset(candp, 0.0)
            for r, b in enumerate(MED_BITS):
                nc.vector.tensor_tensor(
                    out=candp[0:1, :].bitcast(u32),
                    in0=kb[:, 32 * r:32 * r + 32],
                    in1=P.to_broadcast((1, 32)), op=Alu.add)
                nc.vector.transpose(out=candT, in_=candp)
                nc.vector.tensor_scalar(
                    out=cjunk, in0=e_rep, scalar1=candT[0:32, 0:1],
                    scalar2=0.0, op0=Alu.is_lt, op1=Alu.add,
                    accum_out=cntp[0:32, 0:1])
                nc.vector.transpose(out=cntT, in_=cntp)
                nc.vector.tensor_scalar(
                    out=cntT[0:1, 0:31], in0=cntT[0:1, 0:31], scalar1=1024.5,
                    scalar2=0.0, op0=Alu.is_lt, op1=Alu.add, accum_out=dd)
                nc.vector.tensor_scalar(out=stepf, in0=dd,
                                        scalar1=float(1 << b), scalar2=None,
                                        op0=Alu.mult)
                nc.vector.tensor_copy(out=stepu, in_=stepf)
                nc.vector.tensor_tensor(out=P, in0=P, in1=stepu, op=Alu.add)
            nc.gpsimd.partition_broadcast(out_ap=den42[s], in_ap=P.bitcast(f32))
            nc.vector.tensor_scalar(out=den42[s], in0=den42[s],
                                    scalar1=1.0e-6, scalar2=None, op0=Alu.add)

        # ================= inverse =================
        def inv(s, pool_ok):
            ebins = all_ebins[s]
            xb = all_xb[s]
            x0f = all_x0f[s]
            ths = msk_p.tile([42, 51], f32, tag="ths")
            nc.vector.tensor_scalar(out=ths, in0=thrp, scalar1=den42[s],
                                    scalar2=None, op0=Alu.mult)
            hardP = msk_p.tile([64, 64], f32, tag="hardP")
            nc.vector.memset(hardP, 0.0)
            nc.vector.tensor_tensor(out=hardP[0:42, 0:51], in0=ebins,
                                    in1=ths, op=Alu.is_gt)
            mTf = msk_p.tile([64, 64], f32, tag="mTf")
            for a in range(2):
                for bb in range(2):
                    nc.vector.transpose(
                        out=mTf[32 * bb:32 * bb + 32, 32 * a:32 * a + 32],
                        in_=hardP[32 * a:32 * a + 32, 32 * bb:32 * bb + 32])
            # dense group-mask [116, 41]: cols 0..39 = groups, col 40 unused
            mT2b = msk_p.tile([116, 41], bf16, tag="mT2b")
            nc.vector.memset(mT2b, 0.0)
            nc.vector.tensor_copy(out=mT2b[0:51, 0:32], in_=mTf[0:51, 0:32])
            nc.vector.tensor_copy(out=mT2b[0:51, 32:40], in_=mTf[0:51, 33:41])
            nc.vector.tensor_copy(out=mT2b[64:115, 0:32], in_=mTf[0:51, 0:32])
            nc.vector.tensor_copy(out=mT2b[64:115, 32:40], in_=mTf[0:51, 33:41])
            m0c = msk_p.tile([42, 1], bf16, tag="m0c")
            nc.vector.memset(m0c, 0.0)
            nc.vector.tensor_copy(out=m0c[0:9, 0:1], in_=mTf[0:9, 32:33])
            nc.vector.tensor_copy(out=m0c[32:41, 0:1], in_=mTf[0:9, 32:33])

            st0f = t_p.tile([128, FW], f16, tag="t0", name="st0f")
            st1f = t_p.tile([114, FW], f16, tag="t1", name="st1f")
            st0 = st0f.bitcast(bf16)
            st1 = st1f.bitcast(bf16)
            xbv = xb.rearrange("p (g c) -> p g c", g=40, c=256)

            for f5 in range(5):  # 8 groups per fifth
                g0 = 8 * f5
                cc = cc_p.tile([116, 8 * 512], bf16, tag="cc")
                ccv = cc.rearrange("p (h g c) -> p h g c", h=2, g=8, c=256)
                maskb1 = mT2b[:, g0:g0 + 8, None].broadcast_to(
                    (116, 8, 256))
                wdbc = wd_v[:, :, None, :].broadcast_to((116, 2, 8, 256))
                for h in range(2):
                    whb1 = whd_v[:, h, None, :].broadcast_to((116, 8, 256))
                    nc.vector.tensor_tensor(out=ccv[:, h], in0=maskb1,
                                            in1=whb1, op=Alu.mult)
                nc.vector.tensor_tensor(out=ccv, in0=ccv, in1=wdbc,
                                        op=Alu.add)
                nc.vector.tensor_tensor(out=cc[:, 0:2048],
                                        in0=xb[:, 2048 * f5:2048 * f5 + 2048],
                                        in1=cc[:, 0:2048], op=Alu.mult)
                nc.vector.tensor_tensor(out=cc[:, 2048:4096],
                                        in0=xb[:, 2048 * f5:2048 * f5 + 2048],
                                        in1=cc[:, 2048:4096], op=Alu.mult)
                for hq in range(2):
                    sps = pA.tile([128, 1024], f32, tag="pa")
                    for k in range(4):
                        g = g0 + 4 * hq + k
                        nc.tensor.matmul(
                            sps[0:102, 256 * k:256 * k + 256],
                            ainv_sb[:, 102 * g:102 * g + 102],
                            ccv[:, 0, 4 * hq + k, :], start=True, stop=False)
                        nc.tensor.matmul(
                            sps[0:102, 256 * k:256 * k + 256],
                            ainvs_sb[:, 102 * g:102 * g + 102],
                            ccv[:, 1, 4 * hq + k, :], start=False, stop=True)
                    ssb = ssb_p.tile([102, 1024], bf16, tag="ssb")
                    evict(ssb, sps[0:102, :])
                    for k in range(4):
                        g = g0 + 4 * hq + k
                        ctdst = (st0[6 * g + 1:6 * g + 7, :] if g <= 20 else
                                 st1[6 * (g - 21):6 * (g - 21) + 6, :]
                                 ).rearrange("i (q c) -> i q c", q=17, c=256)
                        ct_eng(pool_ok).dma_start(
                            out=ctdst, in_=ssb[:, 256 * k:256 * k + 256])

            # class 0
            cre0 = y0_p.tile([42, 256], bf16, tag="cre0")
            cim0 = y0_p.tile([42, 256], bf16, tag="cim0")
            nc.vector.scalar_tensor_tensor(
                out=cre0, in0=whd[0:42, 0:256], scalar=m0c,
                in1=wd[0:42, 0:256], op0=Alu.mult, op1=Alu.add)
            nc.vector.scalar_tensor_tensor(
                out=cim0, in0=whd[0:42, 256:512], scalar=m0c,
                in1=wd[0:42, 256:512], op0=Alu.mult, op1=Alu.add)
            nc.vector.tensor_tensor(out=cre0, in0=x0f, in1=cre0, op=Alu.mult)
            nc.vector.tensor_tensor(out=cim0, in0=x0f, in1=cim0, op=Alu.mult)
            s0ps = pA.tile([128, 1024], f32, tag="pa")
            nc.tensor.matmul(s0ps[0:18, 0:256], ainv0_sb, cre0,
                             start=True, stop=False)
            nc.tensor.matmul(s0ps[0:18, 0:256], ainv0s_sb, cim0,
                             start=False, stop=True)
            s0sb = ssb_p.tile([18, 256], bf16, tag="sc0")
            nc.scalar.copy(out=s0sb, in_=s0ps[0:18, 0:256])
            nc.sync.dma_start(
                out=st0[0:1, :].rearrange("i (q c) -> i q c", q=17, c=256),
                in_=s0sb[0:17, :])

            # stage B, column-half major; output DMA per (colhalf, mt)
            ov = out_t.ap().rearrange("s (a b) c -> s a b c", a=241, b=17)
            for ch in range(2):
                pr = EPAIRS[0:3] if ch == 0 else EPAIRS[3:5]
                wtot = 2304 if ch == 0 else 2048
                lo_h = 0 if ch == 0 else 2304
                osb0 = osb_p.tile([128, 2304], bf16, tag="osb0")
                osb1 = osb_p.tile([113, 2304], bf16, tag="osb1")
                for mt in range(2):
                    dst, rows = (osb0, 128) if mt == 0 else (osb1, 113)
                    for pair in pr:
                        ps = pB.tile([128, 1024], f32, tag="pb")
                        off = 0
                        for ci in pair:
                            lo, w = CHUNKS[ci]
                            nc.tensor.matmul(ps[:, off:off + w],
                                             b1k0[:, 128 * mt:128 * mt + 128],
                                             st0[0:127, lo:lo + w],
                                             start=True, stop=False)
                            nc.tensor.matmul(ps[:, off:off + w],
                                             b1k1[:, 128 * mt:128 * mt + 128],
                                             st1[:, lo:lo + w],
                                             start=False, stop=True)
                            off += w
                        lo0 = CHUNKS[pair[0]][0] - lo_h
                        evict(dst[0:rows, lo0:lo0 + off], ps[0:rows, 0:off])
                    b_lo, b_n = (0, 9) if ch == 0 else (9, 8)
                    oeng = (nc.sync, nc.scalar)[(ch + mt) % 2]
                    oeng.dma_start(
                        out=ov[s:s + 1, 128 * mt:128 * mt + rows,
                               b_lo:b_lo + b_n, :],
                        in_=dst[0:rows, 0:wtot].rearrange(
                            "p (q c) -> p q c", q=b_n, c=256))

        # ================= pipeline =================
        loads(0)
        fwd(0, True)
        loads(1)
        fwd(1, True)
        loads(2)
        median(0)
        median(1)
        fwd(2, False)
        loads(3)
        median(2)
        inv(0, False)
        inv(1, False)
        fwd(3, False)
        median(3)
        inv(2, True)
        inv(3, True)

    nc.compile()
    _NC_CACHE["nc"] = nc
    return nc


def _make_in_maps(x_in, complex_weight, complex_weight_high, threshold_param):
    bf = ml_dtypes.bfloat16
    thrp = np.asarray(threshold_param, np.float32)[_BINM.reshape(-1)]
    thrp = np.ascontiguousarray(thrp.reshape(42, 51))
    cw = np.asarray(complex_weight, np.float32)
    cwh = np.asarray(complex_weight_high, np.float32)
    whd = np.zeros((116, 512), np.float32)
    whd[:, 0:256] = cwh[:, 0]
    whd[:, 256:512] = cwh[:, 1]
    wdm = np.zeros((116, 512), np.float32)
    wdm[:, 0:256] = cw[:, 0]
    wdm[:, 256:512] = cw[:, 1]
    whd = whd.astype(bf)
    wdm = wdm.astype(bf)

    x_in = np.ascontiguousarray(np.asarray(x_in, np.float32))
    in_maps = []
    for core in range(8):
        m = {"x": x_in[BL * core:BL * core + BL],
             "thrp": thrp, "WHD": whd, "WD": wdm}
        m.update(_CONSTS)
        in_maps.append(m)
    return in_maps


def kernel(x_in, complex_weight, complex_weight_high, threshold_param):
    from concourse.bass_utils import run_bass_kernel_spmd
    nc = _build_nc()
    in_maps = _make_in_maps(x_in, complex_weight, complex_weight_high,
                            threshold_param)
    res = run_bass_kernel_spmd(nc, in_maps, core_ids=list(range(8)))
    out = np.concatenate([np.asarray(res.results[i]["out"], np.float32)
                          for i in range(8)], axis=0)
    return out


# revision 5
# speedup vs baseline: 1.0243x; 1.0006x over previous
"""Adaptive Spectral Block on 8 TRN2 NeuronCores (data-parallel over batch).

v2 of the matmul-FFT kernel (N = 4097 = 241*17 Cooley-Tukey):
  - median via radix-16 select run entirely on the gpsimd/pool engine
    (partition_broadcast / partition_all_reduce), overlapped with fwd/inv
    compute of other samples; batches (0,1), (2), (3).
  - pointwise spectral stage via broadcast-AP big tensor ops (CRECIM
    interleaved dup layout built per 8-group fifth, products in place).
  - energy = square(xb bf16) on ACT + grouped tensor_reduce on DVE
    (replaces 41 ACT square+accum ops per sample).
  - PSUM-pair evictions ([128,1024] tiles, 2 chunks/4 groups per evict).
  - corner-turn DMAs spread across sync/scalar/gpsimd queues.
  - output written bf16 (within rel-err budget), 4 big output DMAs.
  - pipeline order f0 f1 M01 f2 M2 i0 i1 f3 M3 i2 i3 keeps PE busy and
    only ~2.5 xb spectra alive (SBUF).
"""
import numpy as np
import ml_dtypes

B, N, C = 32, 4097, 256
F = N // 2 + 1
BL = B // 8
NSQ = np.sqrt(np.float64(N))
FW = 17 * C  # 4352
GW = 40 * C  # 10240

# stage1/stageB column chunking: halves (2304 | 2048), chunks per half
CHUNKS = [(0, 512), (512, 512), (1024, 512), (1536, 512), (2048, 256),
          (2304, 512), (2816, 512), (3328, 512), (3840, 512)]
# eviction pairs (indices into CHUNKS): [(0,1),(2,3),(4,),(5,6),(7,8)]
EPAIRS = [(0, 1), (2, 3), (4,), (5, 6), (7, 8)]
MED_BITS = [26, 21, 16, 11]


def _build_consts():
    n1 = np.arange(241)
    k1 = np.arange(121)
    n2 = np.arange(17)
    k2 = np.arange(17)

    ang = 2 * np.pi * np.outer(n1, k1) / 241.0
    A1 = np.zeros((241, 256), np.float64)
    A1[:, 0:121] = np.cos(ang)
    A1[:, 128:248] = -np.sin(ang[:, 1:121])
    A1 /= NSQ

    def cls_mat(c):
        kk = c + 241 * k2
        th = -2 * np.pi * np.outer(n2, kk) / N
        Cm, Sm = np.cos(th), np.sin(th)
        M = np.zeros((34, 34))
        M[0:17, 0:17] = Cm
        M[17:34, 0:17] = -Sm
        M[0:17, 17:34] = Sm
        M[17:34, 17:34] = Cm
        return M

    sgn = np.ones((40, 51), np.float64)
    binm = np.zeros((42, 51), np.int64)
    for g in range(40):
        rg = g + (1 if g >= 32 else 0)
        for i in range(3):
            c = 3 * g + 1 + i
            for q in range(17):
                k = c + 241 * q
                binm[rg, 17 * i + q] = k if k <= 2048 else N - k
                if k > 2048:
                    sgn[g, 17 * i + q] = -1.0
    for q in range(9):
        binm[32, q] = 241 * q

    A2_all = np.zeros((102, 40, 116), np.float64)
    for g in range(40):
        for i in range(3):
            c = 3 * g + 1 + i
            M = cls_mat(c)
            A2_all[17 * i:17 * i + 17, g, 17 * i:17 * i + 17] = M[0:17, 0:17]
            A2_all[51 + 17 * i:51 + 17 * i + 17, g, 17 * i:17 * i + 17] = M[17:34, 0:17]
            A2_all[17 * i:17 * i + 17, g, 64 + 17 * i:64 + 17 * i + 17] = M[0:17, 17:34]
            A2_all[51 + 17 * i:51 + 17 * i + 17, g, 64 + 17 * i:64 + 17 * i + 17] = M[17:34, 17:34]
    for g in range(40):
        A2_all[:, g, 64:115] *= sgn[g][None, :]
    A2f = A2_all.reshape(102, 40 * 116)

    kk0 = 241 * np.arange(9)
    th0 = -2 * np.pi * np.outer(n2, kk0) / N
    A2_0 = np.zeros((17, 42), np.float64)
    A2_0[:, 0:9] = np.cos(th0)
    A2_0[:, 32:41] = np.sin(th0)

    def cls_inv(c):
        kk = c + 241 * k2
        th = +2 * np.pi * np.outer(n2, kk) / N
        Cm, Sm = np.cos(th), np.sin(th)
        M = np.zeros((34, 34))
        M[0:17, 0:17] = Cm.T
        M[17:34, 0:17] = -Sm.T
        M[0:17, 17:34] = Sm.T
        M[17:34, 17:34] = Cm.T
        return M / NSQ

    Ainv_all = np.zeros((116, 40, 116), np.float64)
    for g in range(40):
        for i in range(3):
            c = 3 * g + 1 + i
            M = cls_inv(c)
            Ainv_all[17 * i:17 * i + 17, g, 17 * i:17 * i + 17] = M[0:17, 0:17]
            Ainv_all[64 + 17 * i:64 + 17 * i + 17, g, 17 * i:17 * i + 17] = M[17:34, 0:17]
            Ainv_all[17 * i:17 * i + 17, g, 64 + 17 * i:64 + 17 * i + 17] = M[0:17, 17:34]
            Ainv_all[64 + 17 * i:64 + 17 * i + 17, g, 64 + 17 * i:64 + 17 * i + 17] = M[17:34, 17:34]

    AINV2 = Ainv_all.copy()
    AINVSWP = np.zeros_like(Ainv_all)
    for g in range(40):
        AINV2[64:115, g, :] = Ainv_all[64:115, g, :] * sgn[g][:, None]
        AINVSWP[0:51, g, :] = AINV2[64:115, g, :]
        AINVSWP[64:115, g, :] = -Ainv_all[0:51, g, :]
    AINV2 = AINV2.reshape(116, 40 * 116)
    AINVSWP = AINVSWP.reshape(116, 40 * 116)

    th = 2 * np.pi * np.outer(np.arange(9), n2) / 17.0
    Ainv0 = np.zeros((42, 18), np.float64)
    Ainv0[0, 0:17] = 1.0
    Ainv0[1:9, 0:17] = 2 * np.cos(th[1:9])
    Ainv0[33:41, 0:17] = -2 * np.sin(th[1:9])
    Ainv0 /= NSQ
    AINV0SWP = np.zeros_like(Ainv0)
    AINV0SWP[0:9, :] = Ainv0[32:41, :]
    AINV0SWP[32:41, :] = -Ainv0[0:9, :]

    ang2 = 2 * np.pi * np.outer(k1, n1) / 241.0
    ck = np.where(k1 == 0, 1.0, 2.0)
    cosr = ck[:, None] * np.cos(ang2)
    sinr = -2.0 * np.sin(ang2[1:121])
    B1 = np.zeros((241, 256), np.float64)
    B1[0:121, 0:128] = cosr[:, 0:128]
    B1[0:121, 128:241] = cosr[:, 128:241]
    B1[121:128, 0:128] = sinr[0:7, 0:128]
    B1[121:128, 128:241] = sinr[0:7, 128:241]
    B1[128:241, 0:128] = sinr[7:120, 0:128]
    B1[128:241, 128:241] = sinr[7:120, 128:241]

    # class-interleaved permutations (same as baseline)
    A1P = np.zeros((241, 256), np.float64)
    A1P[:, 0] = A1[:, 0]
    for c in range(1, 64):
        A1P[:, 2 * c - 1] = A1[:, c]
        A1P[:, 2 * c] = A1[:, 127 + c]
    for c in range(64, 121):
        A1P[:, 128 + 2 * (c - 64)] = A1[:, c]
        A1P[:, 128 + 2 * (c - 64) + 1] = A1[:, 127 + c]

    oldidx = np.zeros(102, np.int64)
    for i in range(3):
        for q in range(17):
            oldidx[34 * i + q] = 17 * i + q
            oldidx[34 * i + 17 + q] = 51 + 17 * i + q
    A2P = A2f[oldidx]

    mold = np.zeros(102, np.int64)
    for i in range(3):
        for q in range(17):
            mold[34 * i + q] = 17 * i + q
            mold[34 * i + 17 + q] = 64 + 17 * i + q
    AINVP = AINV2.reshape(116, 40, 116)[:, :, mold].reshape(116, 40 * 102)
    AINVSP = AINVSWP.reshape(116, 40, 116)[:, :, mold].reshape(116, 40 * 102)

    B1P = np.zeros((241, 256), np.float64)
    B1P[0] = B1[0]
    for c in range(1, 64):
        B1P[2 * c - 1] = B1[c]
        B1P[2 * c] = B1[120 + c]
    for c in range(64, 121):
        B1P[127 + 2 * (c - 64)] = B1[c]
        B1P[127 + 2 * (c - 64) + 1] = B1[120 + c]

    # radix-32 candidate increments: KB[0, 32*r + k] = (k+1) << MED_BITS[r]
    KB = np.zeros((1, 32 * len(MED_BITS)), np.uint32)
    for r, b in enumerate(MED_BITS):
        for k in range(31):
            KB[0, 32 * r + k] = np.uint32((k + 1) << b)
        KB[0, 32 * r + 31] = np.uint32(31 << b)

    bf = ml_dtypes.bfloat16
    f16 = np.float16
    return {
        "A1": A1P.astype(f16), "A2": A2P.astype(f16),
        "A20": A2_0.astype(f16), "AINV": AINVP.astype(bf),
        "AINVS": AINVSP.astype(bf),
        "AINV0": Ainv0.astype(bf), "AINV0S": AINV0SWP.astype(bf),
        "B1": B1P.astype(bf), "KB": KB,
    }, binm


_CONSTS, _BINM = _build_consts()
_NC_CACHE = {}


def _build_nc():
    if "nc" in _NC_CACHE:
        return _NC_CACHE["nc"]
    from contextlib import ExitStack
    from concourse import bacc, tile, mybir
    from concourse import bass_isa
    f32 = mybir.dt.float32
    f16 = mybir.dt.float16
    bf16 = mybir.dt.bfloat16
    u32 = mybir.dt.uint32
    Alu = mybir.AluOpType
    Act = mybir.ActivationFunctionType
    RedOp = bass_isa.ReduceOp

    nc = bacc.Bacc("TRN2", target_bir_lowering=False, debug=False, num_devices=8)
    x_t = nc.dram_tensor("x", [BL, N, C], f32, kind="ExternalInput")
    thr_t = nc.dram_tensor("thrp", [42, 51], f32, kind="ExternalInput")
    whd_t = nc.dram_tensor("WHD", [116, 512], bf16, kind="ExternalInput")
    wd_t = nc.dram_tensor("WD", [116, 512], bf16, kind="ExternalInput")
    a1_t = nc.dram_tensor("A1", [241, 256], f16, kind="ExternalInput")
    a2_t = nc.dram_tensor("A2", [102, 40 * 116], f16, kind="ExternalInput")
    a20_t = nc.dram_tensor("A20", [17, 42], f16, kind="ExternalInput")
    ainv_t = nc.dram_tensor("AINV", [116, 40 * 102], bf16, kind="ExternalInput")
    ainvs_t = nc.dram_tensor("AINVS", [116, 40 * 102], bf16, kind="ExternalInput")
    ainv0_t = nc.dram_tensor("AINV0", [42, 18], bf16, kind="ExternalInput")
    ainv0s_t = nc.dram_tensor("AINV0S", [42, 18], bf16, kind="ExternalInput")
    b1_t = nc.dram_tensor("B1", [241, 256], bf16, kind="ExternalInput")
    kb_t = nc.dram_tensor("KB", [1, 32 * len(MED_BITS)], u32, kind="ExternalInput")
    out_t = nc.dram_tensor("out", [BL, N, C], bf16, kind="ExternalOutput")

    with tile.TileContext(nc) as tc, ExitStack() as ES:
        cpool = ES.enter_context(tc.tile_pool(name="consts", bufs=1))
        xin_p = ES.enter_context(tc.tile_pool(name="xin", bufs=1))
        t_p = ES.enter_context(tc.tile_pool(name="tst", bufs=2))
        mt_p = ES.enter_context(tc.tile_pool(name="mt", bufs=6))
        xb_p = ES.enter_context(tc.tile_pool(name="xb", bufs=3))
        sq_p = ES.enter_context(tc.tile_pool(name="sq", bufs=1))
        e_p = ES.enter_context(tc.tile_pool(name="energy", bufs=2))
        eb_p = ES.enter_context(tc.tile_pool(name="ebins", bufs=1))
        x0_p = ES.enter_context(tc.tile_pool(name="x0f", bufs=1))
        med_p = ES.enter_context(tc.tile_pool(name="med", bufs=1))
        msk_p = ES.enter_context(tc.tile_pool(name="mask", bufs=2))
        cc_p = ES.enter_context(tc.tile_pool(name="crecim", bufs=2))
        ssb_p = ES.enter_context(tc.tile_pool(name="ssb", bufs=3))
        osb_p = ES.enter_context(tc.tile_pool(name="osb", bufs=1))
        y0_p = ES.enter_context(tc.tile_pool(name="cls0", bufs=1))
        pA = ES.enter_context(tc.tile_pool(name="pA", bufs=2, space="PSUM"))
        pB = ES.enter_context(tc.tile_pool(name="pB", bufs=2, space="PSUM"))

        # ---------------- constants ----------------
        a1k0 = cpool.tile([128, 256], f16)
        a1k1 = cpool.tile([113, 256], f16)
        nc.sync.dma_start(out=a1k0, in_=a1_t.ap()[0:128, :])
        nc.sync.dma_start(out=a1k1, in_=a1_t.ap()[128:241, :])
        a2_sb = cpool.tile([102, 40 * 116], f16)
        nc.scalar.dma_start(out=a2_sb, in_=a2_t.ap())
        a20_sb = cpool.tile([17, 42], f16)
        nc.sync.dma_start(out=a20_sb, in_=a20_t.ap())
        ainv_sb = cpool.tile([116, 40 * 102], bf16)
        nc.sync.dma_start(out=ainv_sb, in_=ainv_t.ap())
        ainvs_sb = cpool.tile([116, 40 * 102], bf16)
        nc.scalar.dma_start(out=ainvs_sb, in_=ainvs_t.ap())
        ainv0_sb = cpool.tile([42, 18], bf16)
        nc.sync.dma_start(out=ainv0_sb, in_=ainv0_t.ap())
        ainv0s_sb = cpool.tile([42, 18], bf16)
        nc.sync.dma_start(out=ainv0s_sb, in_=ainv0s_t.ap())
        b1k0 = cpool.tile([127, 256], bf16)
        b1k1 = cpool.tile([114, 256], bf16)
        nc.sync.dma_start(out=b1k0, in_=b1_t.ap()[0:127, :])
        nc.sync.dma_start(out=b1k1, in_=b1_t.ap()[127:241, :])
        whd = cpool.tile([116, 512], bf16)
        wd = cpool.tile([116, 512], bf16)
        nc.sync.dma_start(out=whd, in_=whd_t.ap())
        nc.sync.dma_start(out=wd, in_=wd_t.ap())
        kb = cpool.tile([1, 32 * len(MED_BITS)], u32)
        nc.sync.dma_start(out=kb, in_=kb_t.ap())
        thrp = cpool.tile([42, 51], f32)
        nc.sync.dma_start(out=thrp, in_=thr_t.ap())

        whd_v = whd.rearrange("p (h c) -> p h c", h=2, c=256)
        wd_v = wd.rearrange("p (h c) -> p h c", h=2, c=256)

        all_ebins = [None] * BL
        all_xb = [None] * BL
        all_x0f = [None] * BL
        den42 = [med_p.tile([42, 1], f32, tag=f"den{s}", name=f"den42_{s}")
                 for s in range(BL)]

        ev_cnt = [0]

        def evict(dst, src):
            # rotate PSUM evictions between DVE and ACT
            eng = (nc.vector, nc.scalar)[ev_cnt[0] % 2]
            ev_cnt[0] += 1
            if eng is nc.vector:
                eng.tensor_copy(out=dst, in_=src)
            else:
                eng.copy(out=dst, in_=src)

        ct_cnt = [0]

        def ct_eng(pool_ok):
            engs = (nc.sync, nc.scalar, nc.gpsimd)
            e = engs[ct_cnt[0] % 3]
            ct_cnt[0] += 1
            return e

        # ================= forward =================
        xin_cur = {}

        def loads(s):
            xv = x_t.ap().rearrange("s (a b) c -> s a b c", a=241, b=17)
            xin0a = xin_p.tile([128, 2304], f16, tag="x0a")
            xin1a = xin_p.tile([113, 2304], f16, tag="x1a")
            xin0b = xin_p.tile([128, 2048], f16, tag="x0b")
            xin1b = xin_p.tile([113, 2048], f16, tag="x1b")
            nc.gpsimd.dma_start(out=xin0a, in_=xv[s:s + 1, 0:128, 0:9])
            nc.gpsimd.dma_start(out=xin1a, in_=xv[s:s + 1, 128:241, 0:9])
            nc.gpsimd.dma_start(out=xin0b, in_=xv[s:s + 1, 0:128, 9:17])
            nc.gpsimd.dma_start(out=xin1b, in_=xv[s:s + 1, 128:241, 9:17])
            xin_cur[s] = (xin0a, xin1a, xin0b, xin1b)

        def fwd(s, pool_ok):
            xin0a, xin1a, xin0b, xin1b = xin_cur.pop(s)

            def xin_slice(ci):
                lo, w = CHUNKS[ci]
                if lo < 2304:
                    return (xin0a[:, lo:lo + w], xin1a[:, lo:lo + w])
                lo -= 2304
                return (xin0b[:, lo:lo + w], xin1b[:, lo:lo + w])

            t0 = t_p.tile([128, FW], f16, tag="t0")
            t1 = t_p.tile([114, FW], f16, tag="t1")
            for mt in range(2):
                dst, rows = (t0, 128) if mt == 0 else (t1, 114)
                for pair in EPAIRS:
                    ps = pA.tile([128, 1024], f32, tag="pa")
                    off = 0
                    for ci in pair:
                        lo, w = CHUNKS[ci]
                        x0s, x1s = xin_slice(ci)
                        nc.tensor.matmul(ps[:, off:off + w],
                                         a1k0[:, 128 * mt:128 * mt + 128],
                                         x0s, start=True, stop=False)
                        nc.tensor.matmul(ps[:, off:off + w],
                                         a1k1[:, 128 * mt:128 * mt + 128],
                                         x1s, start=False, stop=True)
                        off += w
                    lo0 = CHUNKS[pair[0]][0]
                    evict(dst[0:rows, lo0:lo0 + off], ps[0:rows, 0:off])

            # corner turn into per-group mt tiles, then stage2
            xb = xb_p.tile([116, GW], bf16, tag="xb")
            all_xb[s] = xb
            e2 = e_p.tile([128, 64], f32, tag="e2")
            nc.vector.memset(e2, 0.0)
            nc.vector.memset(e2[0:128, 41:42], 5.0e29)

            for gq in range(10):  # 4 groups per psum tile
                xps = pB.tile([128, 1024], f32, tag="pb")
                for k in range(4):
                    g = 4 * gq + k
                    mt_g = mt_p.tile([102, 256], f16, tag="m")
                    ctsrc = (t0[6 * g + 1:6 * g + 7, :] if g <= 20 else
                             t1[6 * (g - 21):6 * (g - 21) + 6, :]).rearrange(
                                 "i (q c) -> i q c", q=17, c=256)
                    ct_eng(pool_ok).dma_start(out=mt_g, in_=ctsrc)
                    nc.tensor.matmul(xps[0:116, 256 * k:256 * k + 256],
                                     a2_sb[:, 116 * g:116 * g + 116],
                                     mt_g, start=True, stop=True)
                evict(xb[:, 1024 * gq:1024 * gq + 1024], xps[0:116, :])

            # class 0
            m0 = mt_p.tile([17, 256], f16, tag="mc0")
            nc.sync.dma_start(
                out=m0,
                in_=t0[0:1, :].rearrange("i (q c) -> i q c", q=17, c=256))
            x0ps = pB.tile([128, 1024], f32, tag="pb")
            nc.tensor.matmul(x0ps[0:42, 0:256], a20_sb, m0,
                             start=True, stop=True)
            x0f = x0_p.tile([42, 256], bf16, tag=f"x0f_{s}")
            nc.scalar.copy(out=x0f, in_=x0ps[0:42, 0:256])
            all_x0f[s] = x0f

            # energy: square xb (ACT) then grouped reduce (DVE)
            xbv = xb.rearrange("p (g c) -> p g c", g=40, c=256)
            for f5 in range(5):
                sq = sq_p.tile([116, 2048], bf16, tag="sq")
                sqv = sq.rearrange("p (g c) -> p g c", g=8, c=256)
                nc.scalar.activation(out=sq, in_=xb[:, 2048 * f5:2048 * f5 + 2048],
                                     func=Act.Square)
                rg0 = 8 * f5 + (1 if f5 == 4 else 0)
                nc.vector.tensor_reduce(
                    out=e2[0:116, rg0:rg0 + 8], in_=sqv,
                    axis=mybir.AxisListType.X, op=Alu.add)
            sq0 = sq_p.tile([42, 256], bf16, tag="sq0")
            nc.scalar.activation(out=sq0, in_=x0f, func=Act.Square)
            nc.vector.tensor_reduce(
                out=e2[0:42, 32:33], in_=sq0[:, None, :],
                axis=mybir.AxisListType.X, op=Alu.add)

            e2T = e_p.tile([64, 128], f32, tag="e2T")
            for a in range(4):
                for bb in range(2):
                    nc.vector.transpose(
                        out=e2T[32 * bb:32 * bb + 32, 32 * a:32 * a + 32],
                        in_=e2[32 * a:32 * a + 32, 32 * bb:32 * bb + 32])
            ebins = eb_p.tile([42, 51], f32, tag=f"eb{s}")
            nc.vector.tensor_add(ebins[0:42, 0:51], e2T[0:42, 0:51],
                                 e2T[0:42, 64:115])
            nc.vector.memset(ebins[32:33, 9:51], 1.0e30)
            nc.vector.tensor_add(ebins[32:33, 0:9], ebins[32:33, 0:9],
                                 e2T[32:33, 32:41])
            all_ebins[s] = ebins

        # ========== median (radix-32 select, DVE + pool broadcast) ==========
        def median(s):
            e_rep = med_p.tile([32, 2142], f32, tag="erep")
            cjunk = med_p.tile([32, 2142], bf16, tag="cjunk")
            candp = med_p.tile([32, 32], f32, tag="candp")
            candT = med_p.tile([32, 32], f32, tag="candT")
            cntp = med_p.tile([32, 32], f32, tag="cntp")
            cntT = med_p.tile([32, 32], f32, tag="cntT")
            P = med_p.tile([1, 1], u32, tag="P")
            dd = med_p.tile([1, 1], f32, tag="dd")
            stepf = med_p.tile([1, 1], f32, tag="stepf")
            stepu = med_p.tile([1, 1], u32, tag="stepu")
            nc.sync.dma_start(out=e_rep[0:1, :], in_=all_ebins[s])
            nc.gpsimd.partition_broadcast(out_ap=e_rep, in_ap=e_rep[0:1, :])
            nc.vector.memset(P, 0)
            nc.vector.memset(candp, 0.0)
            for r, b in enumerate(MED_BITS):
                nc.vector.tensor_tensor(
                    out=candp[0:1, :].bitcast(u32),
                    in0=kb[:, 32 * r:32 * r + 32],
                    in1=P.to_broadcast((1, 32)), op=Alu.add)
                nc.vector.transpose(out=candT, in_=candp)
                nc.vector.tensor_scalar(
                    out=cjunk, in0=e_rep, scalar1=candT[0:32, 0:1],
                    scalar2=0.0, op0=Alu.is_lt, op1=Alu.add,
                    accum_out=cntp[0:32, 0:1])
                nc.vector.transpose(out=cntT, in_=cntp)
                nc.vector.tensor_scalar(
                    out=cntT[0:1, 0:31], in0=cntT[0:1, 0:31], scalar1=1024.5,
                    scalar2=0.0, op0=Alu.is_lt, op1=Alu.add, accum_out=dd)
                nc.vector.tensor_scalar(out=stepf, in0=dd,
                                        scalar1=float(1 << b), scalar2=None,
                                        op0=Alu.mult)
                nc.vector.tensor_copy(out=stepu, in_=stepf)
                nc.vector.tensor_tensor(out=P, in0=P, in1=stepu, op=Alu.add)
            nc.gpsimd.partition_broadcast(out_ap=den42[s], in_ap=P.bitcast(f32))
            nc.vector.tensor_scalar(out=den42[s], in0=den42[s],
                                    scalar1=1.0e-6, scalar2=None, op0=Alu.add)

        # ================= inverse =================
        def inv(s, pool_ok):
            ebins = all_ebins[s]
            xb = all_xb[s]
            x0f = all_x0f[s]
            ths = msk_p.tile([42, 51], f32, tag="ths")
            nc.vector.tensor_scalar(out=ths, in0=thrp, scalar1=den42[s],
                                    scalar2=None, op0=Alu.mult)
            hardP = msk_p.tile([64, 64], f32, tag="hardP")
            nc.vector.memset(hardP, 0.0)
            nc.vector.tensor_tensor(out=hardP[0:42, 0:51], in0=ebins,
                                    in1=ths, op=Alu.is_gt)
            mTf = msk_p.tile([64, 64], f32, tag="mTf")
            for a in range(2):
                for bb in range(2):
                    nc.vector.transpose(
                        out=mTf[32 * bb:32 * bb + 32, 32 * a:32 * a + 32],
                        in_=hardP[32 * a:32 * a + 32, 32 * bb:32 * bb + 32])
            # dense group-mask [116, 41]: cols 0..39 = groups, col 40 unused
            mT2b = msk_p.tile([116, 41], bf16, tag="mT2b")
            nc.vector.memset(mT2b, 0.0)
            nc.vector.tensor_copy(out=mT2b[0:51, 0:32], in_=mTf[0:51, 0:32])
            nc.vector.tensor_copy(out=mT2b[0:51, 32:40], in_=mTf[0:51, 33:41])
            nc.vector.tensor_copy(out=mT2b[64:115, 0:32], in_=mTf[0:51, 0:32])
            nc.vector.tensor_copy(out=mT2b[64:115, 32:40], in_=mTf[0:51, 33:41])
            m0c = msk_p.tile([42, 1], bf16, tag="m0c")
            nc.vector.memset(m0c, 0.0)
            nc.vector.tensor_copy(out=m0c[0:9, 0:1], in_=mTf[0:9, 32:33])
            nc.vector.tensor_copy(out=m0c[32:41, 0:1], in_=mTf[0:9, 32:33])

            st0f = t_p.tile([128, FW], f16, tag="t0", name="st0f")
            st1f = t_p.tile([114, FW], f16, tag="t1", name="st1f")
            st0 = st0f.bitcast(bf16)
            st1 = st1f.bitcast(bf16)
            xbv = xb.rearrange("p (g c) -> p g c", g=40, c=256)

            for f5 in range(5):  # 8 groups per fifth
                g0 = 8 * f5
                cc = cc_p.tile([116, 8 * 512], bf16, tag="cc")
                ccv = cc.rearrange("p (h g c) -> p h g c", h=2, g=8, c=256)
                for g in range(8):
                    nc.vector.scalar_tensor_tensor(
                        out=ccv[:, :, g, :], in0=whd_v,
                        scalar=mT2b[:, g0 + g:g0 + g + 1], in1=wd_v,
                        op0=Alu.mult, op1=Alu.add)
                nc.vector.tensor_tensor(out=cc[:, 0:2048],
                                        in0=xb[:, 2048 * f5:2048 * f5 + 2048],
                                        in1=cc[:, 0:2048], op=Alu.mult)
                nc.vector.tensor_tensor(out=cc[:, 2048:4096],
                                        in0=xb[:, 2048 * f5:2048 * f5 + 2048],
                                        in1=cc[:, 2048:4096], op=Alu.mult)
                for hq in range(2):
                    sps = pA.tile([128, 1024], f32, tag="pa")
                    for k in range(4):
                        g = g0 + 4 * hq + k
                        nc.tensor.matmul(
                            sps[0:102, 256 * k:256 * k + 256],
                            ainv_sb[:, 102 * g:102 * g + 102],
                            ccv[:, 0, 4 * hq + k, :], start=True, stop=False)
                        nc.tensor.matmul(
                            sps[0:102, 256 * k:256 * k + 256],
                            ainvs_sb[:, 102 * g:102 * g + 102],
                            ccv[:, 1, 4 * hq + k, :], start=False, stop=True)
                    ssb = ssb_p.tile([102, 1024], bf16, tag="ssb")
                    evict(ssb, sps[0:102, :])
                    for k in range(4):
                        g = g0 + 4 * hq + k
                        ctdst = (st0[6 * g + 1:6 * g + 7, :] if g <= 20 else
                                 st1[6 * (g - 21):6 * (g - 21) + 6, :]
                                 ).rearrange("i (q c) -> i q c", q=17, c=256)
                        ct_eng(pool_ok).dma_start(
                            out=ctdst, in_=ssb[:, 256 * k:256 * k + 256])

            # class 0
            cre0 = y0_p.tile([42, 256], bf16, tag="cre0")
            cim0 = y0_p.tile([42, 256], bf16, tag="cim0")
            nc.vector.scalar_tensor_tensor(
                out=cre0, in0=whd[0:42, 0:256], scalar=m0c,
                in1=wd[0:42, 0:256], op0=Alu.mult, op1=Alu.add)
            nc.vector.scalar_tensor_tensor(
                out=cim0, in0=whd[0:42, 256:512], scalar=m0c,
                in1=wd[0:42, 256:512], op0=Alu.mult, op1=Alu.add)
            nc.vector.tensor_tensor(out=cre0, in0=x0f, in1=cre0, op=Alu.mult)
            nc.vector.tensor_tensor(out=cim0, in0=x0f, in1=cim0, op=Alu.mult)
            s0ps = pA.tile([128, 1024], f32, tag="pa")
            nc.tensor.matmul(s0ps[0:18, 0:256], ainv0_sb, cre0,
                             start=True, stop=False)
            nc.tensor.matmul(s0ps[0:18, 0:256], ainv0s_sb, cim0,
                             start=False, stop=True)
            s0sb = ssb_p.tile([18, 256], bf16, tag="sc0")
            nc.scalar.copy(out=s0sb, in_=s0ps[0:18, 0:256])
            nc.sync.dma_start(
                out=st0[0:1, :].rearrange("i (q c) -> i q c", q=17, c=256),
                in_=s0sb[0:17, :])

            # stage B, column-half major; output DMA per (colhalf, mt)
            ov = out_t.ap().rearrange("s (a b) c -> s a b c", a=241, b=17)
            for ch in range(2):
                pr = EPAIRS[0:3] if ch == 0 else EPAIRS[3:5]
                wtot = 2304 if ch == 0 else 2048
                lo_h = 0 if ch == 0 else 2304
                osb0 = osb_p.tile([128, 2304], bf16, tag="osb0")
                osb1 = osb_p.tile([113, 2304], bf16, tag="osb1")
                for mt in range(2):
                    dst, rows = (osb0, 128) if mt == 0 else (osb1, 113)
                    for pair in pr:
                        ps = pB.tile([128, 1024], f32, tag="pb")
                        off = 0
                        for ci in pair:
                            lo, w = CHUNKS[ci]
                            nc.tensor.matmul(ps[:, off:off + w],
                                             b1k0[:, 128 * mt:128 * mt + 128],
                                             st0[0:127, lo:lo + w],
                                             start=True, stop=False)
                            nc.tensor.matmul(ps[:, off:off + w],
                                             b1k1[:, 128 * mt:128 * mt + 128],
                                             st1[:, lo:lo + w],
                                             start=False, stop=True)
                            off += w
                        lo0 = CHUNKS[pair[0]][0] - lo_h
                        evict(dst[0:rows, lo0:lo0 + off], ps[0:rows, 0:off])
                    b_lo, b_n = (0, 9) if ch == 0 else (9, 8)
                    oeng = (nc.sync, nc.scalar)[(ch + mt) % 2]
                    oeng.dma_start(
                        out=ov[s:s + 1, 128 * mt:128 * mt + rows,
                               b_lo:b_lo + b_n, :],
                        in_=dst[0:rows, 0:wtot].rearrange(
                            "p (q c) -> p q c", q=b_n, c=256))

        # ================= pipeline =================
        loads(0)
        fwd(0, True)
        loads(1)
        fwd(1, True)
        loads(2)
        median(0)
        median(1)
        fwd(2, False)
        loads(3)
        median(2)
        inv(0, False)
        inv(1, False)
        fwd(3, False)
        median(3)
        inv(2, True)
        inv(3, True)

    nc.compile()
    _NC_CACHE["nc"] = nc
    return nc


def _make_in_maps(x_in, complex_weight, complex_weight_high, threshold_param):
    bf = ml_dtypes.bfloat16
    thrp = np.asarray(threshold_param, np.float32)[_BINM.reshape(-1)]
    thrp = np.ascontiguousarray(thrp.reshape(42, 51))
    cw = np.asarray(complex_weight, np.float32)
    cwh = np.asarray(complex_weight_high, np.float32)
    whd = np.zeros((116, 512), np.float32)
    whd[:, 0:256] = cwh[:, 0]
    whd[:, 256:512] = cwh[:, 1]
    wdm = np.zeros((116, 512), np.float32)
    wdm[:, 0:256] = cw[:, 0]
    wdm[:, 256:512] = cw[:, 1]
    whd = whd.astype(bf)
    wdm = wdm.astype(bf)

    x_in = np.ascontiguousarray(np.asarray(x_in, np.float32))
    in_maps = []
    for core in range(8):
        m = {"x": x_in[BL * core:BL * core + BL],
             "thrp": thrp, "WHD": whd, "WD": wdm}
        m.update(_CONSTS)
        in_maps.append(m)
    return in_maps


def kernel(x_in, complex_weight, complex_weight_high, threshold_param):
    from concourse.bass_utils import run_bass_kernel_spmd
    nc = _build_nc()
    in_maps = _make_in_maps(x_in, complex_weight, complex_weight_high,
                            threshold_param)
    res = run_bass_kernel_spmd(nc, in_maps, core_ids=list(range(8)))
    out = np.concatenate([np.asarray(res.results[i]["out"], np.float32)
                          for i in range(8)], axis=0)
    return out
